# revision 24
# baseline (speedup 1.0000x reference)
"""Trainium2 Bass kernel for nn_Attention_6133213298828.

Batch-parallel multi-head attention with per-query-position relative-position
logits, forward pass only. Data-parallel over 8 NeuronCores (batch dim);
weights replicated, no collectives.

Per-core design (2048 batches, 16 chunks of 128 batches / 2176 tokens):
  - All three x-consuming projections (QK, rel, V) run as compensated fp8e4
    DoubleRow matmuls: 3 half-rate matmuls (W_hi*x_hi + (W_hi/16)*(16*x_lo)
    + W_lo*x_hi) replace 2 full-rate fp16 matmuls, ~25% less PE streaming at
    ~0.15% rel error. x ships as fp8 hi + fp8(16*lo) pairs in the DoubleRow
    fold-100 layout (K=200 logical rows: 192 features + 8 static mask rows).
    Weights are scaled into fp8 normal range (q: 64x, k/v: 32x, wrel: 512x);
    the scale cancels via exp(score * 2^-11) and W_out/32. All mask/one-hot
    constants are chosen exactly representable in fp8/f16 (and <= 448, the
    e4m3 max: the -480 ones-row constant of an earlier revision became fp8
    NaN and poisoned everything).
  - Attention on 119-token groups (7 batches x 17 positions <= 128
    partitions); relative logits fold into the score matmul as 26 extra
    contraction rows (one-hot key rows against per-token rel projections);
    cross-batch garbage killed by an additive -30*2048 mask generated by
    static indicator rows riding the fp8 x tensor.
  - Softmax denominators via a ones column in V; normalization multiplies
    the AV PSUM rows by a broadcast reciprocal during eviction; AV output is
    PE-transposed back to feature-major for the output projection.
  - Engine balance (the kernel is eviction-bound on Act+DVE, not PE-bound):
    Act runs exp + most QK/out-proj evictions; DVE runs frm/V/normalize
    evictions, transposes eviction, and 2 of the 15 QK eviction windows;
    the k_h2 re-basing copy runs on the otherwise-idle Pool (gpsimd) engine
    (SBUF->SBUF only - gpsimd cannot touch PSUM); input DMA on the gpsimd
    SWDGE queue, output DMA on sync/HWDGE, avoiding head-of-line blocking
    between loads and stores on one queue.
"""

import numpy as np

DIM, OUT_DIM, H, V, B = 192, 192, 3, 17, 16384
DK = DIM // H
NCORES = 8
BC = B // NCORES          # batches per core
NB = 128                  # batches per chunk
NCHUNK = BC // NB         # 16
TC = NB * V               # 2176 tokens per chunk
TOK = BC * V              # 34816 tokens per core
GSIZES = [119] * 18 + [34]            # token-group sizes within a chunk
GOFFS = np.cumsum([0] + GSIZES).tolist()
G = len(GSIZES)           # 19
NGH = G * H               # 57 (group, head) tiles per chunk
SCALE = DIM ** -0.5
SW = 32.0                 # fp8 weight scale (k/v)
SQ = 64.0                 # fp8 weight scale (q); scores at 2048x
SREL = 512.0              # fp8 wrel scale
ONEHOT = 4.0              # eml one-hot value = 2048/SREL
C1 = 192.0                # frm mask-row magnitude (exact fp8, <= 448)
C2 = 320.0                # eml mask-row magnitude; C1*C2 = 61440 = 30*2048
C3 = -240.0               # frm ones-row value (exact fp8, <= 448)
C4 = 256.0                # eml ones-row value; C3*C4 = -61440
EXPSCALE = float(2.0 ** -11)

KL = 200                  # logical contraction rows: 192 features + 8 mask
KP = 100                  # DoubleRow partitions
USE_DMAT = False          # DMA (XBAR) transpose vs PE transpose for aot
DEBUG_DUMP = False        # add chunk-0 intermediate dumps as outputs

_CACHED = {}


def _fp8(a):
    import ml_dtypes
    return a.astype(ml_dtypes.float8_e4m3fn)


def _f32(a):
    return np.asarray(a, np.float32)


def _pack_dr(a):
    """[KL, M] -> DoubleRow fold-100 layout [KP, 2*M]: (p, s*M+m) = a[s*KP+p, m]."""
    m = a.shape[1]
    return np.ascontiguousarray(
        a.reshape(2, KP, m).transpose(1, 0, 2).reshape(KP, 2 * m))


def _build_host_constants(W_qkv, b_qkv, key_rel, key_rel_diag, W_out, b_out):
    f16 = np.float16
    scale = np.float32(SCALE)

    # QK projection weights, q columns pre-scaled; slab order
    # slabA = [q^h0; q^h1], slabB = [k_h0; k_h1], slabC = [q^h2; k_h2].
    qs = W_qkv[:, 0:DIM] * (scale * SQ)
    kk = W_qkv[:, DIM:2 * DIM] * SW
    wqk = np.concatenate(
        [qs[:, 0:128], kk[:, 0:128], qs[:, 128:192], kk[:, 128:192]], axis=1)
    wv = W_qkv[:, 2 * DIM:3 * DIM] * SW

    # KRABS[i, j] = relative key vector seen by query position i at absolute
    # key position j (diag vector on j == i).
    kr = key_rel.reshape(V, V - 1, DK)
    KRABS = np.zeros((V, V, DK), np.float32)
    for i in range(V):
        for j in range(V):
            KRABS[i, j] = key_rel_diag[0] if j == i else kr[i, j - (j > i)]

    # wrel[i]: (192, 96). Columns 32h + j' (j' < 17) hold
    # SREL * scale * W_q[:, head h] @ KRABS[i, j'].
    wrel = np.zeros((V, DIM, 96), np.float32)
    for h in range(H):
        wq_h = W_qkv[:, h * DK:(h + 1) * DK]          # (192, 64)
        proj = np.einsum('dk,ijk->dij', wq_h, KRABS) * (scale * SREL)
        for i in range(V):
            wrel[i, :, 32 * h:32 * h + 17] = proj[:, i, :]

    # Static patterns over a chunk's 2176 tokens.
    t = np.arange(TC)
    pos = t % V               # position within sequence
    grp = (t // V) % 7        # batch index within 119-token group
    # eml: one-hot key rows (ONEHOT = 2048/SREL) + mask rows.
    eml = np.zeros((26, TC), np.float32)
    for j in range(V):
        eml[j] = ONEHOT * (pos == j)
    for a in range(7):
        eml[17 + a] = C2 * (grp == a)
    eml[24] = 0.0
    eml[25] = C4
    emlp = np.concatenate([eml, np.zeros((6, TC), np.float32)], axis=0)
    eml3 = np.concatenate([emlp, emlp, emlp], axis=0)   # (96, TC)

    # mgrp: static rows 192:200 of the fp8 x tensor (hi only).
    mgrp = np.zeros((8, TC), np.float32)
    for a in range(7):
        mgrp[a] = (grp == a)
    mgrp[7] = 1.0

    # wrel extended to KL rows: rows 192:200 hold the mask-generation
    # constants (exact in fp8): row 192+a pairs with mgrp[a] to emit C1 into
    # frm's mask rows; row 199 pairs with the ones row to emit C3.
    wrel_ext = np.zeros((KL, V * 96), np.float32)
    wrel_ext[0:DIM] = wrel.transpose(1, 0, 2).reshape(DIM, V * 96)
    for h in range(H):
        for a in range(7):
            for i in range(V):
                wrel_ext[192 + a, i * 96 + 32 * h + 17 + a] = C1
        for i in range(V):
            wrel_ext[199, i * 96 + 32 * h + 25] = C3

    wqk_ext = np.zeros((KL, 384), np.float32)
    wqk_ext[0:DIM] = wqk
    wv_ext = np.zeros((KL, 192), np.float32)
    wv_ext[0:DIM] = wv

    def dr_variants(ext):
        hi = _f32(_fp8(ext))
        lo = _f32(_fp8(ext - hi))
        hi16 = _f32(_fp8(hi / 16.0))
        return (_pack_dr(_fp8(hi)), _pack_dr(_fp8(hi16)), _pack_dr(_fp8(lo)))

    qk_hi, qk_hi16, qk_lo = dr_variants(wqk_ext)
    v_hi, v_hi16, v_lo = dr_variants(wv_ext)
    r_hi, r_hi16, r_lo = dr_variants(wrel_ext)

    consts = {
        "wqk_hi": qk_hi, "wqk_hi16": qk_hi16, "wqk_lo": qk_lo,
        "wv_hi": v_hi, "wv_hi16": v_hi16, "wv_lo": v_lo,
        "wrel_hi": r_hi, "wrel_hi16": r_hi16, "wrel_lo": r_lo,
        "wout0": (W_out[0:128] / SW).astype(f16),
        "wout1": (W_out[128:192] / SW).astype(f16),
        "eml": eml3.astype(f16),
        "ident": np.eye(128, dtype=f16),
    }
    return consts, mgrp


def _build_bass():
    import concourse.bacc as bacc
    import concourse.mybir as mybir
    from concourse import tile

    f16 = mybir.dt.float16
    f32 = mybir.dt.float32
    f8 = mybir.dt.float8e4
    EXP = mybir.ActivationFunctionType.Exp
    MUL = mybir.AluOpType.mult
    DR = mybir.MatmulPerfMode.DoubleRow

    nc = bacc.Bacc(None, target_bir_lowering=False)

    def dp(name, shape, dt=f16):
        return nc.declare_dram_parameter(name, list(shape), dt, isOutput=False)

    x_hi_in = dp("x_hi", (KP, 2 * TOK), f8)
    x_lo_in = dp("x_lo", (KP, 2 * TOK), f8)
    wqk_d = [dp(f"wqk_{s}", (KP, 2 * 384), f8) for s in ("hi", "hi16", "lo")]
    wv_d = [dp(f"wv_{s}", (KP, 2 * 192), f8) for s in ("hi", "hi16", "lo")]
    wrel_d = [dp(f"wrel_{s}", (KP, 2 * V * 96), f8) for s in ("hi", "hi16", "lo")]
    wout0_d = dp("wout0", (128, 192))
    wout1_d = dp("wout1", (64, 192))
    eml_d = dp("eml", (96, TC))
    ident_d = dp("ident", (128, 128))
    y_out = nc.declare_dram_parameter("y", [TOK, DIM], f32, isOutput=True)
    dbg = {}
    if DEBUG_DUMP:
        for nm, shp, dt in [("d_qka", (128, TC), f16), ("d_qkb", (128, TC), f16),
                            ("d_qkc", (128, TC), f16), ("d_frm", (96, TC), f16),
                            ("d_attn", (119, NGH * 119), f16),
                            ("d_vt", (119, G * 195), f16),
                            ("d_avout", (119, G * 192), f16),
                            ("d_aotA", (128, TC), f16), ("d_aotB", (128, TC), f16)]:
            dbg[nm] = nc.declare_dram_parameter(nm, list(shp), dt, isOutput=True)

    NT512 = [(0, 512), (512, 512), (1024, 512), (1536, 512), (2048, 128)]

    from contextlib import ExitStack
    with tile.TileContext(nc) as tc, ExitStack() as es:
        wp = es.enter_context(tc.sbuf_pool(name="wpool", bufs=1))
        sp = es.enter_context(tc.sbuf_pool(name="work", bufs=2))
        dpool = es.enter_context(tc.tile_pool(name="dsc", space="DRAM", bufs=2))
        psE = es.enter_context(tc.psum_pool(name="psE", bufs=3))
        psL = es.enter_context(tc.psum_pool(name="psL", bufs=3))
        pst = None if USE_DMAT else es.enter_context(tc.psum_pool(name="pst", bufs=1))
        if True:

            # ---- persistent weights ----
            wqk8 = []
            for s, d in zip(("hi", "hi16", "lo"), wqk_d):
                t8 = wp.tile([KP, 2 * 384], f8, name=f"wqk8_{s}")
                nc.sync.dma_start(out=t8[:], in_=d[:])
                wqk8.append(t8[:].rearrange("p (s m) -> p s m", s=2))
            wv8 = []
            for s, d in zip(("hi", "hi16", "lo"), wv_d):
                t8 = wp.tile([KP, 2 * 192], f8, name=f"wv8_{s}")
                nc.sync.dma_start(out=t8[:], in_=d[:])
                wv8.append(t8[:].rearrange("p (s m) -> p s m", s=2))
            wrel8 = []
            for s, d in zip(("hi", "hi16", "lo"), wrel_d):
                t8 = wp.tile([KP, 2 * V * 96], f8, name=f"wrel8_{s}")
                nc.sync.dma_start(out=t8[:], in_=d[:])
                wrel8.append(t8[:].rearrange("p (s i m) -> p s i m", s=2, m=96))
            wout0 = wp.tile([128, 192], f16)
            nc.sync.dma_start(out=wout0[:], in_=wout0_d[:])
            woutB = wp.tile([128, 192], f16)
            nc.sync.dma_start(out=woutB[64:128, :], in_=wout1_d[:])
            eml = wp.tile([96, TC], f16)
            nc.sync.dma_start(out=eml[:], in_=eml_d[:])
            ident = wp.tile([128, 128], f16)
            nc.sync.dma_start(out=ident[:], in_=ident_d[:])

            prev = {}

            def emit_po(pv):
                # out-projection + store for a completed (transposed) chunk
                aotA, aotB, r0p = pv["aotA"], pv["aotB"], pv["r0"]
                fin = sp.tile([128, 17 * 192], f32, tag="fin")
                finv = fin[:].rearrange("p (t c) -> p t c", c=192)
                for tp in range(9):          # packs of 2 token-tiles
                    npo = min(2, 17 - tp * 2)
                    po = psL.tile([128, 512], f32, tag="psL")
                    for u in range(npo):
                        t = tp * 2 + u
                        gc = slice(t * 128, t * 128 + 128)
                        nc.tensor.matmul(po[:, u * 256:u * 256 + 192],
                                         aotA[:, gc], wout0[:],
                                         start=True, stop=False)
                        nc.tensor.matmul(po[:, u * 256:u * 256 + 192],
                                         aotB[64:128, gc], woutB[64:128, :],
                                         start=False, stop=True)
                    dst = finv[:, tp * 2:tp * 2 + npo, :]
                    src_ = po[:, 0:npo * 256].rearrange(
                        "p (u c) -> p u c", c=256)[:, :, 0:192]
                    if tp % 2 == 0:
                        nc.scalar.copy(dst, src_)
                    else:
                        nc.vector.tensor_copy(dst, src_)
                for s0, s1 in ((0, 6), (6, 12), (12, 17)):
                    nc.scalar.dma_start(
                        out=y_out[r0p + s0 * 128:r0p + s1 * 128, :].rearrange(
                            "(t p) d -> p t d", p=128),
                        in_=fin[:, s0 * 192:s1 * 192].rearrange(
                            "p (t d) -> p t d", d=192))

            for c in range(NCHUNK):
                r0 = c * TC
                # ---- fp8 x loads (DoubleRow fold-100 layout) ----
                xhi = sp.tile([KP, 2 * TC], f8, tag="xhi", bufs=3)
                xlo = sp.tile([KP, 2 * TC], f8, tag="xlo", bufs=3)
                nc.gpsimd.dma_start(
                    out=xhi[:].rearrange("p (s t) -> p s t", s=2),
                    in_=x_hi_in[:].rearrange("p (s t) -> p s t", s=2)[:, :, r0:r0 + TC])
                nc.gpsimd.dma_start(
                    out=xlo[:].rearrange("p (s t) -> p s t", s=2),
                    in_=x_lo_in[:].rearrange("p (s t) -> p s t", s=2)[:, :, r0:r0 + TC])
                if prev:
                    emit_po(prev)
                xhiv = xhi[:].rearrange("p (s t) -> p s t", s=2)
                xlov = xlo[:].rearrange("p (s t) -> p s t", s=2)
                xhip = xhi[:].rearrange("p (s b v) -> p s b v", s=2, v=V)
                xlop = xlo[:].rearrange("p (s b v) -> p s b v", s=2, v=V)

                # ---- rel projections -> frm (96, TC), packs of 8 positions ----
                frm = sp.tile([96, TC], f16, tag="frm", bufs=3)
                frmv = frm[:].rearrange("p (b v) -> p b v", v=V)
                for ip in range(5):          # packs of 4 positions
                    n = min(4, V - ip * 4)
                    pr = psE.tile([128, 512], f32, tag="psE")
                    for u in range(n):
                        i = ip * 4 + u
                        o = u * 128
                        nc.tensor.matmul(pr[0:96, o:o + 128],
                                         wrel8[0][:, :, i, :], xhip[:, :, :, i],
                                         start=True, stop=False, perf_mode=DR)
                        nc.tensor.matmul(pr[0:96, o:o + 128],
                                         wrel8[1][:, :, i, :], xlop[:, :, :, i],
                                         start=False, stop=False, perf_mode=DR)
                        nc.tensor.matmul(pr[0:96, o:o + 128],
                                         wrel8[2][:, :, i, :], xhip[:, :, :, i],
                                         start=False, stop=True, perf_mode=DR)
                    nc.vector.tensor_copy(
                        frmv[:, :, ip * 4:ip * 4 + n],
                        pr[0:96, 0:n * 128].rearrange("p (i b) -> p b i", b=128))

                # ---- QK^T projections -> 3 slabs ----
                qka = sp.tile([128, TC], f16, tag="qka")
                qkb = sp.tile([128, TC], f16, tag="qkb")
                qkc = sp.tile([128, TC], f16, tag="qkc")
                slabs = [qka, qkb, qkc]
                kh2t = sp.tile([64, TC], f16, tag="kh2t")
                ei = 0
                for m in (2, 0, 1):
                    for n0, nw in NT512:
                        pq = psE.tile([128, 512], f32, tag="psE")
                        nc.tensor.matmul(pq[:, 0:nw],
                                         wqk8[0][:, :, m * 128:(m + 1) * 128],
                                         xhiv[:, :, n0:n0 + nw],
                                         start=True, stop=False, perf_mode=DR)
                        nc.tensor.matmul(pq[:, 0:nw],
                                         wqk8[1][:, :, m * 128:(m + 1) * 128],
                                         xlov[:, :, n0:n0 + nw],
                                         start=False, stop=False, perf_mode=DR)
                        nc.tensor.matmul(pq[:, 0:nw],
                                         wqk8[2][:, :, m * 128:(m + 1) * 128],
                                         xhiv[:, :, n0:n0 + nw],
                                         start=False, stop=True, perf_mode=DR)
                        if ei % 2 == 0:
                            nc.scalar.copy(slabs[m][:, n0:n0 + nw], pq[:, 0:nw])
                        else:
                            nc.vector.tensor_copy(slabs[m][:, n0:n0 + nw],
                                                  pq[:, 0:nw])
                        ei += 1
                        if m == 2:
                            nc.vector.tensor_copy(kh2t[:, n0:n0 + nw],
                                                  qkc[64:128, n0:n0 + nw])

                # ---- dots^T + rel + mask, exp; packs of 8 (g,h) tiles ----
                QT = [qka[0:64, :], qka[64:128, :], qkc[0:64, :]]
                KT = [qkb[0:64, :], qkb[64:128, :], kh2t[0:64, :]]
                attn = sp.tile([119, NGH * 119], f16, tag="attn")
                for pk in range(15):         # packs of 4 (g,h) tiles
                    n = min(4, NGH - pk * 4)
                    pd = psE.tile([128, 512], f32, tag="psE")
                    for u in range(n):
                        idx = pk * 4 + u
                        g, h = divmod(idx, H)
                        gs = GSIZES[g]
                        gc = slice(GOFFS[g], GOFFS[g] + gs)
                        o = u * 128
                        nc.tensor.matmul(pd[0:gs, o:o + gs], KT[h][:, gc],
                                         QT[h][:, gc], start=True, stop=False)
                        nc.tensor.matmul(pd[0:gs, o:o + gs],
                                         eml[32 * h:32 * h + 26, gc],
                                         frm[32 * h:32 * h + 26, gc],
                                         start=False, stop=True)
                    pr_rows = 119 if n > 1 else GSIZES[-1]
                    nc.scalar.activation(
                        attn[0:pr_rows, pk * 476:pk * 476 + n * 119].rearrange(
                            "p (u c) -> p u c", c=119),
                        pd[0:pr_rows, 0:n * 128].rearrange(
                            "p (u c) -> p u c", c=128)[:, :, 0:119],
                        EXP, scale=EXPSCALE)

                # ---- V projection (token-major, +ones column) ----
                vt = sp.tile([119, G * 195], f16, tag="vt")
                nc.gpsimd.memset(
                    vt[:].rearrange("p (g hh c) -> p g hh c", hh=3, c=65)[:, :, :, 64:65],
                    1.0)
                vtv = vt[:].rearrange("p (g hh c) -> p g hh c", hh=3, c=65)
                for gp in range(10):         # packs of 2 groups
                    n = min(2, G - gp * 2)
                    pv = psL.tile([128, 512], f32, tag="psL")
                    for u in range(n):
                        g = gp * 2 + u
                        gs = GSIZES[g]
                        gc = slice(GOFFS[g], GOFFS[g] + gs)
                        nc.tensor.matmul(pv[0:gs, u * 256:u * 256 + 192],
                                         xhiv[:, :, gc], wv8[0],
                                         start=True, stop=False, perf_mode=DR)
                        nc.tensor.matmul(pv[0:gs, u * 256:u * 256 + 192],
                                         xlov[:, :, gc], wv8[1],
                                         start=False, stop=False, perf_mode=DR)
                        nc.tensor.matmul(pv[0:gs, u * 256:u * 256 + 192],
                                         xhiv[:, :, gc], wv8[2],
                                         start=False, stop=True, perf_mode=DR)
                    g0 = gp * 2
                    vr = 119 if n > 1 else GSIZES[-1]
                    nc.vector.tensor_copy(
                        vtv[0:vr, g0:g0 + n, :, 0:64],
                        pv[0:vr, 0:n * 256].rearrange(
                            "p (u hh c) -> p u hh c", hh=4, c=64)[:, :, 0:3, :])

                # ---- attention @ V (+denominator), normalize on eviction ----
                avout = sp.tile([119, G * 192], f16, tag="avout")
                avv = avout[:].rearrange("p (g hh c) -> p g hh c", hh=3, c=64)
                recip = sp.tile([119, NGH], f32, tag="recip")
                recv = recip[:].rearrange("p (g hh) -> p g hh", hh=3)
                for gp in range(10):         # packs of 2 groups
                    n = min(2, G - gp * 2)
                    pa = psL.tile([128, 512], f32, tag="psL")
                    for u in range(n):
                        g = gp * 2 + u
                        gs = GSIZES[g]
                        for h in range(H):
                            idx = g * H + h
                            nc.tensor.matmul(
                                pa[0:gs, u * 256 + 65 * h:u * 256 + 65 * h + 65],
                                attn[0:gs, idx * 119:idx * 119 + gs],
                                vtv[0:gs, g, h, :],
                                start=True, stop=True)
                    g0 = gp * 2
                    ar = 119 if n > 1 else GSIZES[-1]
                    pav = pa[0:ar, 0:n * 256].rearrange(
                        "p (u q) -> p u q", q=256)[:, :, 0:195].rearrange(
                        "p u (hh c) -> p u hh c", c=65)
                    nc.vector.reciprocal(recv[0:ar, g0:g0 + n, :], pav[:, :, :, 64])
                    nc.vector.tensor_tensor(
                        avv[0:ar, g0:g0 + n, :, :],
                        pav[:, :, :, 0:64],
                        recv[0:ar, g0:g0 + n, :].broadcast_to([ar, n, 3, 64]),
                        MUL)

                # ---- feature-major via DMA transpose (XBAR) ----
                aotA = sp.tile([128, TC], f16, tag="aotA")
                aotB = sp.tile([128, TC], f16, tag="aotB")
                if USE_DMAT:
                    sc = dpool.tile([TC, 192], f16, tag="dsc")
                    nc.sync.dma_start(
                        out=sc[0:10 * 119, :].rearrange("(g p) d -> p g d", p=119),
                        in_=avout[0:119, 0:10 * 192].rearrange("p (g d) -> p g d", d=192))
                    nc.sync.dma_start(
                        out=sc[10 * 119:18 * 119, :].rearrange("(g p) d -> p g d", p=119),
                        in_=avout[0:119, 10 * 192:18 * 192].rearrange("p (g d) -> p g d", d=192))
                    nc.sync.dma_start(out=sc[18 * 119:TC, :],
                                      in_=avout[0:34, 18 * 192:19 * 192])
                    nc.sync.dma_start_transpose(aotA[:], sc[:, 0:128])
                    nc.sync.dma_start_transpose(aotB[:], sc[:, 64:192])
                else:
                    # PE transposes (baseline path): avout [119, g, 192] ->
                    # aotA feats 0:128, aotB rows 64:128 = feats 128:192.
                    for gp in range(5):          # packs of 4 groups
                        n = min(4, G - gp * 4)
                        pc = pst.tile([128, 512], f16, tag="pst")
                        pe = pst.tile([128, 512], f16, tag="pst")
                        for u in range(n):
                            g = gp * 4 + u
                            gs = GSIZES[g]
                            nc.tensor.transpose(pc[:, u * 128:u * 128 + gs],
                                                avout[0:gs, g * 192:g * 192 + 128],
                                                ident[0:gs, 0:gs])
                            nc.tensor.transpose(pe[0:64, u * 128:u * 128 + gs],
                                                avout[0:gs, g * 192 + 128:g * 192 + 192],
                                                ident[0:gs, 0:gs])
                        t0 = GOFFS[gp * 4]
                        if n == 4 and GSIZES[gp * 4 + 3] == 119:
                            nc.vector.tensor_copy(
                                aotA[:, t0:t0 + 476].rearrange("p (u c) -> p u c", c=119),
                                pc[:, :].rearrange("p (u c) -> p u c", c=128)[:, 0:4, 0:119])
                            nc.vector.tensor_copy(
                                aotB[64:128, t0:t0 + 476].rearrange("p (u c) -> p u c", c=119),
                                pe[0:64, :].rearrange("p (u c) -> p u c", c=128)[:, 0:4, 0:119])
                        else:
                            for u in range(n):
                                g = gp * 4 + u
                                gs = GSIZES[g]
                                gt = GOFFS[g]
                                nc.vector.tensor_copy(aotA[:, gt:gt + gs],
                                                      pc[:, u * 128:u * 128 + gs])
                                nc.vector.tensor_copy(aotB[64:128, gt:gt + gs],
                                                      pe[0:64, u * 128:u * 128 + gs])

                prev = {"aotA": aotA, "aotB": aotB, "r0": r0}

            emit_po(prev)

    nc.finalize()
    return nc


def kernel(x, W_qkv, b_qkv, key_rel, key_rel_diag, W_out, b_out):
    from concourse.bass_utils import run_bass_kernel_spmd

    x = np.ascontiguousarray(np.asarray(x, dtype=np.float32))
    consts, mgrp = _build_host_constants(
        _f32(W_qkv), _f32(b_qkv), _f32(key_rel), _f32(key_rel_diag),
        _f32(W_out), _f32(b_out))

    if "nc" not in _CACHED:
        _CACHED["nc"] = _build_bass()
    nc = _CACHED["nc"]

    # x_ext: [NCORES, KL, TOK] logical rows = 192 features + 8 mask rows,
    # then hi/lo split and DoubleRow fold-100 packing.
    xT = x.reshape(NCORES, BC * V, DIM).transpose(0, 2, 1)  # [NC, 192, TOK]
    mgrp_full = np.tile(mgrp, (1, NCHUNK))                  # [8, TOK]
    x_hi_l = []
    x_lo_l = []
    for k in range(NCORES):
        ext = np.concatenate([xT[k], mgrp_full], axis=0)    # [200, TOK]
        hi = _f32(_fp8(ext))
        lo = (ext - hi) * 16.0
        lo[DIM:KL] = 0.0
        x_hi_l.append(_pack_dr(_fp8(hi)))
        x_lo_l.append(_pack_dr(_fp8(lo)))

    in_maps = [dict(consts, x_hi=x_hi_l[k], x_lo=x_lo_l[k])
               for k in range(NCORES)]
    res = run_bass_kernel_spmd(nc, in_maps, core_ids=list(range(NCORES)))
    _CACHED["last_result"] = res
    out = np.stack([res.results[k]["y"] for k in range(NCORES)], axis=0)
    out = out.reshape(B, V, DIM)
    # Value bias passes through normalized attention as a constant; fold it
    # (and b_out) exactly on the host. The q/k biases are zeros per the spec.
    bv = _f32(b_qkv)[2 * DIM:3 * DIM]
    bias = bv @ _f32(W_out) + _f32(b_out)
    if np.any(bias):
        out = out + bias
    return out


# revision 25
# speedup vs baseline: 1.1039x; 1.1039x over previous
"""Trainium2 Bass kernel for nn_Attention_6133213298828.

Batch-parallel multi-head attention with per-query-position relative-position
logits, forward pass only. Data-parallel over 8 NeuronCores (batch dim);
weights replicated, no collectives.

Per-core design (2048 batches, 16 chunks of 128 batches / 2176 tokens):
  - All three x-consuming projections (QK, rel, V) run as compensated fp8e4
    DoubleRow matmuls: 3 half-rate matmuls (W_hi*x_hi + (W_hi/16)*(16*x_lo)
    + W_lo*x_hi) replace 2 full-rate fp16 matmuls, ~25% less PE streaming at
    ~0.15% rel error. x ships as fp8 hi + fp8(16*lo) pairs in the DoubleRow
    fold-100 layout (K=200 logical rows: 192 features + 8 static mask rows).
    Weights are scaled into fp8 normal range (q: 64x, k/v: 32x, wrel: 512x);
    the scale cancels via exp(score * 2^-11) and W_out/32. All mask/one-hot
    constants are chosen exactly representable in fp8/f16 (and <= 448, the
    e4m3 max: the -480 ones-row constant of an earlier revision became fp8
    NaN and poisoned everything).
  - Attention on 119-token groups (7 batches x 17 positions <= 128
    partitions); relative logits fold into the score matmul as 26 extra
    contraction rows (one-hot key rows against per-token rel projections);
    cross-batch garbage killed by an additive -30*2048 mask generated by
    static indicator rows riding the fp8 x tensor.
  - Softmax denominators via a ones column in V; normalization multiplies
    the AV PSUM rows by a broadcast reciprocal during eviction; AV output is
    PE-transposed back to feature-major for the output projection.
  - Engine balance (the kernel is eviction-bound on Act+DVE, not PE-bound):
    Act runs exp + most QK/out-proj evictions; DVE runs frm/V/normalize
    evictions, transposes eviction, and 2 of the 15 QK eviction windows;
    the k_h2 re-basing copy runs on the otherwise-idle Pool (gpsimd) engine
    (SBUF->SBUF only - gpsimd cannot touch PSUM); input DMA on the gpsimd
    SWDGE queue, output DMA on sync/HWDGE, avoiding head-of-line blocking
    between loads and stores on one queue.
"""

import numpy as np

DIM, OUT_DIM, H, V, B = 192, 192, 3, 17, 16384
DK = DIM // H
NCORES = 8
BC = B // NCORES          # batches per core
NB = 128                  # batches per chunk
NCHUNK = BC // NB         # 16
TC = NB * V               # 2176 tokens per chunk
TOK = BC * V              # 34816 tokens per core
GSIZES = [119] * 18 + [34]            # token-group sizes within a chunk
GOFFS = np.cumsum([0] + GSIZES).tolist()
G = len(GSIZES)           # 19
NGH = G * H               # 57 (group, head) tiles per chunk
SCALE = DIM ** -0.5
SW = 32.0                 # fp8 weight scale (k/v)
SQ = 64.0                 # fp8 weight scale (q); scores at 2048x
SREL = 512.0              # fp8 wrel scale
ONEHOT = 4.0              # eml one-hot value = 2048/SREL
C1 = 192.0                # frm mask-row magnitude (exact fp8, <= 448)
C2 = 320.0                # eml mask-row magnitude; C1*C2 = 61440 = 30*2048
C3 = -240.0               # frm ones-row value (exact fp8, <= 448)
C4 = 256.0                # eml ones-row value; C3*C4 = -61440
EXPSCALE = float(2.0 ** -11)

KL = 200                  # logical contraction rows: 192 features + 8 mask
KP = 100                  # DoubleRow partitions
USE_DMAT = False          # DMA (XBAR) transpose vs PE transpose for aot
DEBUG_DUMP = False        # add chunk-0 intermediate dumps as outputs

_CACHED = {}


def _fp8(a):
    import ml_dtypes
    return a.astype(ml_dtypes.float8_e4m3fn)


def _f32(a):
    return np.asarray(a, np.float32)


def _pack_dr(a):
    """[KL, M] -> DoubleRow fold-100 layout [KP, 2*M]: (p, s*M+m) = a[s*KP+p, m]."""
    m = a.shape[1]
    return np.ascontiguousarray(
        a.reshape(2, KP, m).transpose(1, 0, 2).reshape(KP, 2 * m))


def _build_host_constants(W_qkv, b_qkv, key_rel, key_rel_diag, W_out, b_out):
    f16 = np.float16
    scale = np.float32(SCALE)

    # QK projection weights, q columns pre-scaled; slab order
    # slabA = [q^h0; q^h1], slabB = [k_h0; k_h1], slabC = [q^h2; k_h2].
    qs = W_qkv[:, 0:DIM] * (scale * SQ)
    kk = W_qkv[:, DIM:2 * DIM] * SW
    wqk = np.concatenate(
        [qs[:, 0:128], kk[:, 0:128], qs[:, 128:192], kk[:, 128:192]], axis=1)
    wv = W_qkv[:, 2 * DIM:3 * DIM] * SW

    # KRABS[i, j] = relative key vector seen by query position i at absolute
    # key position j (diag vector on j == i).
    kr = key_rel.reshape(V, V - 1, DK)
    KRABS = np.zeros((V, V, DK), np.float32)
    for i in range(V):
        for j in range(V):
            KRABS[i, j] = key_rel_diag[0] if j == i else kr[i, j - (j > i)]

    # wrel[i]: (192, 96). Columns 32h + j' (j' < 17) hold
    # SREL * scale * W_q[:, head h] @ KRABS[i, j'].
    wrel = np.zeros((V, DIM, 96), np.float32)
    for h in range(H):
        wq_h = W_qkv[:, h * DK:(h + 1) * DK]          # (192, 64)
        proj = np.einsum('dk,ijk->dij', wq_h, KRABS) * (scale * SREL)
        for i in range(V):
            wrel[i, :, 32 * h:32 * h + 17] = proj[:, i, :]

    # Static patterns over a chunk's 2176 tokens.
    t = np.arange(TC)
    pos = t % V               # position within sequence
    grp = (t // V) % 7        # batch index within 119-token group
    # eml: one-hot key rows (ONEHOT = 2048/SREL) + mask rows.
    eml = np.zeros((26, TC), np.float32)
    for j in range(V):
        eml[j] = ONEHOT * (pos == j)
    for a in range(7):
        eml[17 + a] = C2 * (grp == a)
    eml[24] = 0.0
    eml[25] = C4
    emlp = np.concatenate([eml, np.zeros((6, TC), np.float32)], axis=0)
    eml3 = np.concatenate([emlp, emlp, emlp], axis=0)   # (96, TC)

    # mgrp: static rows 192:200 of the fp8 x tensor (hi only).
    mgrp = np.zeros((8, TC), np.float32)
    for a in range(7):
        mgrp[a] = (grp == a)
    mgrp[7] = 1.0

    # wrel extended to KL rows: rows 192:200 hold the mask-generation
    # constants (exact in fp8): row 192+a pairs with mgrp[a] to emit C1 into
    # frm's mask rows; row 199 pairs with the ones row to emit C3.
    wrel_ext = np.zeros((KL, V * 96), np.float32)
    wrel_ext[0:DIM] = wrel.transpose(1, 0, 2).reshape(DIM, V * 96)
    for h in range(H):
        for a in range(7):
            for i in range(V):
                wrel_ext[192 + a, i * 96 + 32 * h + 17 + a] = C1
        for i in range(V):
            wrel_ext[199, i * 96 + 32 * h + 25] = C3

    wqk_ext = np.zeros((KL, 384), np.float32)
    wqk_ext[0:DIM] = wqk
    wv_ext = np.zeros((KL, 192), np.float32)
    wv_ext[0:DIM] = wv

    def dr_variants(ext):
        hi = _f32(_fp8(ext))
        lo = _f32(_fp8(ext - hi))
        hi16 = _f32(_fp8(hi / 16.0))
        return (_pack_dr(_fp8(hi)), _pack_dr(_fp8(hi16)), _pack_dr(_fp8(lo)))

    qk_hi, qk_hi16, qk_lo = dr_variants(wqk_ext)
    v_hi, v_hi16, v_lo = dr_variants(wv_ext)
    r_hi, r_hi16, r_lo = dr_variants(wrel_ext)

    consts = {
        "wqk_hi": qk_hi, "wqk_hi16": qk_hi16, "wqk_lo": qk_lo,
        "wv_hi": v_hi, "wv_hi16": v_hi16, "wv_lo": v_lo,
        "wrel_hi": r_hi, "wrel_hi16": r_hi16, "wrel_lo": r_lo,
        "wout0": (W_out[0:128] / SW).astype(f16),
        "wout1": (W_out[128:192] / SW).astype(f16),
        "eml": eml3.astype(f16),
        "ident": np.eye(128, dtype=f16),
    }
    return consts, mgrp


def _build_bass():
    import concourse.bacc as bacc
    import concourse.mybir as mybir
    from concourse import tile

    f16 = mybir.dt.float16
    f32 = mybir.dt.float32
    f8 = mybir.dt.float8e4
    EXP = mybir.ActivationFunctionType.Exp
    MUL = mybir.AluOpType.mult
    DR = mybir.MatmulPerfMode.DoubleRow

    nc = bacc.Bacc(None, target_bir_lowering=False)

    def dp(name, shape, dt=f16):
        return nc.declare_dram_parameter(name, list(shape), dt, isOutput=False)

    x_hi_in = dp("x_hi", (KP, 2 * TOK), f8)
    x_lo_in = dp("x_lo", (KP, 2 * TOK), f8)
    wqk_d = [dp(f"wqk_{s}", (KP, 2 * 384), f8) for s in ("hi", "hi16", "lo")]
    wv_d = [dp(f"wv_{s}", (KP, 2 * 192), f8) for s in ("hi", "hi16", "lo")]
    wrel_d = [dp(f"wrel_{s}", (KP, 2 * V * 96), f8) for s in ("hi", "hi16", "lo")]
    wout0_d = dp("wout0", (128, 192))
    wout1_d = dp("wout1", (64, 192))
    eml_d = dp("eml", (96, TC))
    ident_d = dp("ident", (128, 128))
    y_out = nc.declare_dram_parameter("y", [TOK, DIM], f32, isOutput=True)
    dbg = {}
    if DEBUG_DUMP:
        for nm, shp, dt in [("d_qka", (128, TC), f16), ("d_qkb", (128, TC), f16),
                            ("d_qkc", (128, TC), f16), ("d_frm", (96, TC), f16),
                            ("d_attn", (119, NGH * 119), f16),
                            ("d_vt", (119, G * 195), f16),
                            ("d_avout", (119, G * 192), f16),
                            ("d_aotA", (128, TC), f16), ("d_aotB", (128, TC), f16)]:
            dbg[nm] = nc.declare_dram_parameter(nm, list(shp), dt, isOutput=True)

    NT512 = [(0, 512), (512, 512), (1024, 512), (1536, 512), (2048, 128)]

    from contextlib import ExitStack
    with tile.TileContext(nc) as tc, ExitStack() as es:
        wp = es.enter_context(tc.sbuf_pool(name="wpool", bufs=1))
        sp = es.enter_context(tc.sbuf_pool(name="work", bufs=2))
        dpool = es.enter_context(tc.tile_pool(name="dsc", space="DRAM", bufs=2))
        psE = es.enter_context(tc.psum_pool(name="psE", bufs=3))
        psL = es.enter_context(tc.psum_pool(name="psL", bufs=3))
        pst = None if USE_DMAT else es.enter_context(tc.psum_pool(name="pst", bufs=1))
        if True:

            # ---- persistent weights ----
            wqk8 = []
            for s, d in zip(("hi", "hi16", "lo"), wqk_d):
                t8 = wp.tile([KP, 2 * 384], f8, name=f"wqk8_{s}")
                nc.sync.dma_start(out=t8[:], in_=d[:])
                wqk8.append(t8[:].rearrange("p (s m) -> p s m", s=2))
            wv8 = []
            for s, d in zip(("hi", "hi16", "lo"), wv_d):
                t8 = wp.tile([KP, 2 * 192], f8, name=f"wv8_{s}")
                nc.sync.dma_start(out=t8[:], in_=d[:])
                wv8.append(t8[:].rearrange("p (s m) -> p s m", s=2))
            wrel8 = []
            for s, d in zip(("hi", "hi16", "lo"), wrel_d):
                t8 = wp.tile([KP, 2 * V * 96], f8, name=f"wrel8_{s}")
                nc.sync.dma_start(out=t8[:], in_=d[:])
                wrel8.append(t8[:].rearrange("p (s i m) -> p s i m", s=2, m=96))
            wout0 = wp.tile([128, 192], f16)
            nc.sync.dma_start(out=wout0[:], in_=wout0_d[:])
            woutB = wp.tile([128, 192], f16)
            nc.sync.dma_start(out=woutB[64:128, :], in_=wout1_d[:])
            eml = wp.tile([96, TC], f16)
            nc.sync.dma_start(out=eml[:], in_=eml_d[:])
            ident = wp.tile([128, 128], f16)
            nc.sync.dma_start(out=ident[:], in_=ident_d[:])

            prev = {}

            def emit_po(pv):
                # out-projection + store for a completed (transposed) chunk
                aotA, aotB, r0p = pv["aotA"], pv["aotB"], pv["r0"]
                fin = sp.tile([128, 17 * 192], f32, tag="fin")
                finv = fin[:].rearrange("p (t c) -> p t c", c=192)
                for tp in range(9):          # packs of 2 token-tiles
                    npo = min(2, 17 - tp * 2)
                    po = psL.tile([128, 512], f32, tag="psL")
                    for u in range(npo):
                        t = tp * 2 + u
                        gc = slice(t * 128, t * 128 + 128)
                        nc.tensor.matmul(po[:, u * 256:u * 256 + 192],
                                         aotA[:, gc], wout0[:],
                                         start=True, stop=False)
                        nc.tensor.matmul(po[:, u * 256:u * 256 + 192],
                                         aotB[64:128, gc], woutB[64:128, :],
                                         start=False, stop=True)
                    dst = finv[:, tp * 2:tp * 2 + npo, :]
                    src_ = po[:, 0:npo * 256].rearrange(
                        "p (u c) -> p u c", c=256)[:, :, 0:192]
                    if tp % 2 == 0:
                        nc.scalar.copy(dst, src_)
                    else:
                        nc.vector.tensor_copy(dst, src_)
                for s0, s1 in ((0, 6), (6, 12), (12, 17)):
                    nc.scalar.dma_start(
                        out=y_out[r0p + s0 * 128:r0p + s1 * 128, :].rearrange(
                            "(t p) d -> p t d", p=128),
                        in_=fin[:, s0 * 192:s1 * 192].rearrange(
                            "p (t d) -> p t d", d=192))

            for c in range(NCHUNK):
                r0 = c * TC
                # ---- fp8 x loads (DoubleRow fold-100 layout) ----
                xhi = sp.tile([KP, 2 * TC], f8, tag="xhi", bufs=3)
                xlo = sp.tile([KP, 2 * TC], f8, tag="xlo", bufs=3)
                nc.gpsimd.dma_start(
                    out=xhi[:].rearrange("p (s t) -> p s t", s=2),
                    in_=x_hi_in[:].rearrange("p (s t) -> p s t", s=2)[:, :, r0:r0 + TC])
                nc.gpsimd.dma_start(
                    out=xlo[:].rearrange("p (s t) -> p s t", s=2),
                    in_=x_lo_in[:].rearrange("p (s t) -> p s t", s=2)[:, :, r0:r0 + TC])
                if prev:
                    emit_po(prev)
                xhiv = xhi[:].rearrange("p (s t) -> p s t", s=2)
                xlov = xlo[:].rearrange("p (s t) -> p s t", s=2)
                xhip = xhi[:].rearrange("p (s b v) -> p s b v", s=2, v=V)
                xlop = xlo[:].rearrange("p (s b v) -> p s b v", s=2, v=V)

                # ---- rel projections -> frm (96, TC), packs of 8 positions ----
                frm = sp.tile([96, TC], f16, tag="frm")
                frmv = frm[:].rearrange("p (b v) -> p b v", v=V)
                for ip in range(5):          # packs of 4 positions
                    n = min(4, V - ip * 4)
                    pr = psE.tile([128, 512], f32, tag="psE")
                    for u in range(n):
                        i = ip * 4 + u
                        o = u * 128
                        nc.tensor.matmul(pr[0:96, o:o + 128],
                                         wrel8[0][:, :, i, :], xhip[:, :, :, i],
                                         start=True, stop=False, perf_mode=DR)
                        nc.tensor.matmul(pr[0:96, o:o + 128],
                                         wrel8[1][:, :, i, :], xlop[:, :, :, i],
                                         start=False, stop=False, perf_mode=DR)
                        nc.tensor.matmul(pr[0:96, o:o + 128],
                                         wrel8[2][:, :, i, :], xhip[:, :, :, i],
                                         start=False, stop=True, perf_mode=DR)
                    nc.vector.tensor_copy(
                        frmv[:, :, ip * 4:ip * 4 + n],
                        pr[0:96, 0:n * 128].rearrange("p (i b) -> p b i", b=128))

                # ---- QK^T projections -> 3 slabs ----
                qka = sp.tile([128, TC], f16, tag="qka")
                qkb = sp.tile([128, TC], f16, tag="qkb")
                qkc = sp.tile([128, TC], f16, tag="qkc")
                slabs = [qka, qkb, qkc]
                kh2t = sp.tile([64, TC], f16, tag="kh2t")
                ei = 0
                for m in (2, 0, 1):
                    for n0, nw in NT512:
                        pq = psE.tile([128, 512], f32, tag="psE")
                        nc.tensor.matmul(pq[:, 0:nw],
                                         wqk8[0][:, :, m * 128:(m + 1) * 128],
                                         xhiv[:, :, n0:n0 + nw],
                                         start=True, stop=False, perf_mode=DR)
                        nc.tensor.matmul(pq[:, 0:nw],
                                         wqk8[1][:, :, m * 128:(m + 1) * 128],
                                         xlov[:, :, n0:n0 + nw],
                                         start=False, stop=False, perf_mode=DR)
                        nc.tensor.matmul(pq[:, 0:nw],
                                         wqk8[2][:, :, m * 128:(m + 1) * 128],
                                         xhiv[:, :, n0:n0 + nw],
                                         start=False, stop=True, perf_mode=DR)
                        if ei % 2 == 0:
                            nc.scalar.copy(slabs[m][:, n0:n0 + nw], pq[:, 0:nw])
                        else:
                            nc.vector.tensor_copy(slabs[m][:, n0:n0 + nw],
                                                  pq[:, 0:nw])
                        ei += 1
                        if m == 2:
                            nc.vector.tensor_copy(kh2t[:, n0:n0 + nw],
                                                  qkc[64:128, n0:n0 + nw])

                # ---- dots^T + rel + mask, exp; packs of 8 (g,h) tiles ----
                QT = [qka[0:64, :], qka[64:128, :], qkc[0:64, :]]
                KT = [qkb[0:64, :], qkb[64:128, :], kh2t[0:64, :]]
                attn = sp.tile([119, NGH * 119], f16, tag="attn")
                for pk in range(15):         # packs of 4 (g,h) tiles
                    n = min(4, NGH - pk * 4)
                    pd = psE.tile([128, 512], f32, tag="psE")
                    for u in range(n):
                        idx = pk * 4 + u
                        g, h = divmod(idx, H)
                        gs = GSIZES[g]
                        gc = slice(GOFFS[g], GOFFS[g] + gs)
                        o = u * 128
                        nc.tensor.matmul(pd[0:gs, o:o + gs], KT[h][:, gc],
                                         QT[h][:, gc], start=True, stop=False)
                        nc.tensor.matmul(pd[0:gs, o:o + gs],
                                         eml[32 * h:32 * h + 26, gc],
                                         frm[32 * h:32 * h + 26, gc],
                                         start=False, stop=True)
                    pr_rows = 119 if n > 1 else GSIZES[-1]
                    nc.scalar.activation(
                        attn[0:pr_rows, pk * 476:pk * 476 + n * 119].rearrange(
                            "p (u c) -> p u c", c=119),
                        pd[0:pr_rows, 0:n * 128].rearrange(
                            "p (u c) -> p u c", c=128)[:, :, 0:119],
                        EXP, scale=EXPSCALE)

                # ---- V projection (token-major, +ones column) ----
                vt = sp.tile([119, G * 195], f16, tag="vt")
                nc.gpsimd.memset(
                    vt[:].rearrange("p (g hh c) -> p g hh c", hh=3, c=65)[:, :, :, 64:65],
                    1.0)
                vtv = vt[:].rearrange("p (g hh c) -> p g hh c", hh=3, c=65)
                for gp in range(10):         # packs of 2 groups
                    n = min(2, G - gp * 2)
                    pv = psL.tile([128, 512], f32, tag="psL")
                    for u in range(n):
                        g = gp * 2 + u
                        gs = GSIZES[g]
                        gc = slice(GOFFS[g], GOFFS[g] + gs)
                        nc.tensor.matmul(pv[0:gs, u * 256:u * 256 + 192],
                                         xhiv[:, :, gc], wv8[0],
                                         start=True, stop=False, perf_mode=DR)
                        nc.tensor.matmul(pv[0:gs, u * 256:u * 256 + 192],
                                         xlov[:, :, gc], wv8[1],
                                         start=False, stop=False, perf_mode=DR)
                        nc.tensor.matmul(pv[0:gs, u * 256:u * 256 + 192],
                                         xhiv[:, :, gc], wv8[2],
                                         start=False, stop=True, perf_mode=DR)
                    g0 = gp * 2
                    vr = 119 if n > 1 else GSIZES[-1]
                    nc.vector.tensor_copy(
                        vtv[0:vr, g0:g0 + n, :, 0:64],
                        pv[0:vr, 0:n * 256].rearrange(
                            "p (u hh c) -> p u hh c", hh=4, c=64)[:, :, 0:3, :])

                # ---- attention @ V (+denominator), normalize on eviction ----
                avout = sp.tile([119, G * 192], f16, tag="avout")
                avv = avout[:].rearrange("p (g hh c) -> p g hh c", hh=3, c=64)
                recip = sp.tile([119, NGH], f32, tag="recip")
                recv = recip[:].rearrange("p (g hh) -> p g hh", hh=3)
                for gp in range(10):         # packs of 2 groups
                    n = min(2, G - gp * 2)
                    pa = psL.tile([128, 512], f32, tag="psL")
                    for u in range(n):
                        g = gp * 2 + u
                        gs = GSIZES[g]
                        for h in range(H):
                            idx = g * H + h
                            nc.tensor.matmul(
                                pa[0:gs, u * 256 + 65 * h:u * 256 + 65 * h + 65],
                                attn[0:gs, idx * 119:idx * 119 + gs],
                                vtv[0:gs, g, h, :],
                                start=True, stop=True)
                    g0 = gp * 2
                    ar = 119 if n > 1 else GSIZES[-1]
                    pav = pa[0:ar, 0:n * 256].rearrange(
                        "p (u q) -> p u q", q=256)[:, :, 0:195].rearrange(
                        "p u (hh c) -> p u hh c", c=65)
                    nc.vector.reciprocal(recv[0:ar, g0:g0 + n, :], pav[:, :, :, 64])
                    nc.vector.tensor_tensor(
                        avv[0:ar, g0:g0 + n, :, :],
                        pav[:, :, :, 0:64],
                        recv[0:ar, g0:g0 + n, :].broadcast_to([ar, n, 3, 64]),
                        MUL)

                # ---- feature-major via DMA transpose (XBAR) ----
                aotA = sp.tile([128, TC], f16, tag="aotA")
                aotB = sp.tile([128, TC], f16, tag="aotB")
                if USE_DMAT:
                    sc = dpool.tile([TC, 192], f16, tag="dsc")
                    nc.sync.dma_start(
                        out=sc[0:10 * 119, :].rearrange("(g p) d -> p g d", p=119),
                        in_=avout[0:119, 0:10 * 192].rearrange("p (g d) -> p g d", d=192))
                    nc.sync.dma_start(
                        out=sc[10 * 119:18 * 119, :].rearrange("(g p) d -> p g d", p=119),
                        in_=avout[0:119, 10 * 192:18 * 192].rearrange("p (g d) -> p g d", d=192))
                    nc.sync.dma_start(out=sc[18 * 119:TC, :],
                                      in_=avout[0:34, 18 * 192:19 * 192])
                    nc.sync.dma_start_transpose(aotA[:], sc[:, 0:128])
                    nc.sync.dma_start_transpose(aotB[:], sc[:, 64:192])
                else:
                    # PE transposes (baseline path): avout [119, g, 192] ->
                    # aotA feats 0:128, aotB rows 64:128 = feats 128:192.
                    for gp in range(5):          # packs of 4 groups
                        n = min(4, G - gp * 4)
                        pc = pst.tile([128, 512], f16, tag="pst")
                        pe = pst.tile([128, 512], f16, tag="pst")
                        for u in range(n):
                            g = gp * 4 + u
                            gs = GSIZES[g]
                            nc.tensor.transpose(pc[:, u * 128:u * 128 + gs],
                                                avout[0:gs, g * 192:g * 192 + 128],
                                                ident[0:gs, 0:gs])
                            nc.tensor.transpose(pe[0:64, u * 128:u * 128 + gs],
                                                avout[0:gs, g * 192 + 128:g * 192 + 192],
                                                ident[0:gs, 0:gs])
                        t0 = GOFFS[gp * 4]
                        if n == 4 and GSIZES[gp * 4 + 3] == 119:
                            nc.vector.tensor_copy(
                                aotA[:, t0:t0 + 476].rearrange("p (u c) -> p u c", c=119),
                                pc[:, :].rearrange("p (u c) -> p u c", c=128)[:, 0:4, 0:119])
                            nc.vector.tensor_copy(
                                aotB[64:128, t0:t0 + 476].rearrange("p (u c) -> p u c", c=119),
                                pe[0:64, :].rearrange("p (u c) -> p u c", c=128)[:, 0:4, 0:119])
                        else:
                            for u in range(n):
                                g = gp * 4 + u
                                gs = GSIZES[g]
                                gt = GOFFS[g]
                                nc.vector.tensor_copy(aotA[:, gt:gt + gs],
                                                      pc[:, u * 128:u * 128 + gs])
                                nc.vector.tensor_copy(aotB[64:128, gt:gt + gs],
                                                      pe[0:64, u * 128:u * 128 + gs])

                prev = {"aotA": aotA, "aotB": aotB, "r0": r0}

            emit_po(prev)

    nc.finalize()
    return nc


def kernel(x, W_qkv, b_qkv, key_rel, key_rel_diag, W_out, b_out):
    from concourse.bass_utils import run_bass_kernel_spmd

    x = np.ascontiguousarray(np.asarray(x, dtype=np.float32))
    consts, mgrp = _build_host_constants(
        _f32(W_qkv), _f32(b_qkv), _f32(key_rel), _f32(key_rel_diag),
        _f32(W_out), _f32(b_out))

    if "nc" not in _CACHED:
        _CACHED["nc"] = _build_bass()
    nc = _CACHED["nc"]

    # x_ext: [NCORES, KL, TOK] logical rows = 192 features + 8 mask rows,
    # then hi/lo split and DoubleRow fold-100 packing.
    xT = x.reshape(NCORES, BC * V, DIM).transpose(0, 2, 1)  # [NC, 192, TOK]
    mgrp_full = np.tile(mgrp, (1, NCHUNK))                  # [8, TOK]
    x_hi_l = []
    x_lo_l = []
    for k in range(NCORES):
        ext = np.concatenate([xT[k], mgrp_full], axis=0)    # [200, TOK]
        hi = _f32(_fp8(ext))
        lo = (ext - hi) * 16.0
        lo[DIM:KL] = 0.0
        x_hi_l.append(_pack_dr(_fp8(hi)))
        x_lo_l.append(_pack_dr(_fp8(lo)))

    in_maps = [dict(consts, x_hi=x_hi_l[k], x_lo=x_lo_l[k])
               for k in range(NCORES)]
    res = run_bass_kernel_spmd(nc, in_maps, core_ids=list(range(NCORES)))
    _CACHED["last_result"] = res
    out = np.stack([res.results[k]["y"] for k in range(NCORES)], axis=0)
    out = out.reshape(B, V, DIM)
    # Value bias passes through normalized attention as a constant; fold it
    # (and b_out) exactly on the host. The q/k biases are zeros per the spec.
    bv = _f32(b_qkv)[2 * DIM:3 * DIM]
    bias = bv @ _f32(W_out) + _f32(b_out)
    if np.any(bias):
        out = out + bias
    return out


# revision 26
# speedup vs baseline: 1.1061x; 1.0019x over previous
"""Trainium2 Bass kernel for nn_Attention_6133213298828.

Batch-parallel multi-head attention with per-query-position relative-position
logits, forward pass only. Data-parallel over 8 NeuronCores (batch dim);
weights replicated, no collectives.

Per-core design (2048 batches, 16 chunks of 128 batches / 2176 tokens):
  - All three x-consuming projections (QK, rel, V) run as compensated fp8e4
    DoubleRow matmuls: 3 half-rate matmuls (W_hi*x_hi + (W_hi/16)*(16*x_lo)
    + W_lo*x_hi) replace 2 full-rate fp16 matmuls, ~25% less PE streaming at
    ~0.15% rel error. x ships as fp8 hi + fp8(16*lo) pairs in the DoubleRow
    fold-100 layout (K=200 logical rows: 192 features + 8 static mask rows).
    Weights are scaled into fp8 normal range (q: 64x, k/v: 32x, wrel: 512x);
    the scale cancels via exp(score * 2^-11) and W_out/32. All mask/one-hot
    constants are chosen exactly representable in fp8/f16 (and <= 448, the
    e4m3 max: the -480 ones-row constant of an earlier revision became fp8
    NaN and poisoned everything).
  - Attention on 119-token groups (7 batches x 17 positions <= 128
    partitions); relative logits fold into the score matmul as 26 extra
    contraction rows (one-hot key rows against per-token rel projections);
    cross-batch garbage killed by an additive -30*2048 mask generated by
    static indicator rows riding the fp8 x tensor.
  - Softmax denominators via a ones column in V; normalization multiplies
    the AV PSUM rows by a broadcast reciprocal during eviction; AV output is
    PE-transposed back to feature-major for the output projection.
  - Engine balance (the kernel is eviction-bound on Act+DVE, not PE-bound):
    Act runs exp + most QK/out-proj evictions; DVE runs frm/V/normalize
    evictions, transposes eviction, and 2 of the 15 QK eviction windows;
    the k_h2 re-basing copy runs on the otherwise-idle Pool (gpsimd) engine
    (SBUF->SBUF only - gpsimd cannot touch PSUM); input DMA on the gpsimd
    SWDGE queue, output DMA on sync/HWDGE, avoiding head-of-line blocking
    between loads and stores on one queue.
"""

import numpy as np

DIM, OUT_DIM, H, V, B = 192, 192, 3, 17, 16384
DK = DIM // H
NCORES = 8
BC = B // NCORES          # batches per core
NB = 128                  # batches per chunk
NCHUNK = BC // NB         # 16
TC = NB * V               # 2176 tokens per chunk
TOK = BC * V              # 34816 tokens per core
GSIZES = [119] * 18 + [34]            # token-group sizes within a chunk
GOFFS = np.cumsum([0] + GSIZES).tolist()
G = len(GSIZES)           # 19
NGH = G * H               # 57 (group, head) tiles per chunk
SCALE = DIM ** -0.5
SW = 32.0                 # fp8 weight scale (k/v)
SQ = 64.0                 # fp8 weight scale (q); scores at 2048x
SREL = 512.0              # fp8 wrel scale
ONEHOT = 4.0              # eml one-hot value = 2048/SREL
C1 = 192.0                # frm mask-row magnitude (exact fp8, <= 448)
C2 = 320.0                # eml mask-row magnitude; C1*C2 = 61440 = 30*2048
C3 = -240.0               # frm ones-row value (exact fp8, <= 448)
C4 = 256.0                # eml ones-row value; C3*C4 = -61440
EXPSCALE = float(2.0 ** -11)

KL = 200                  # logical contraction rows: 192 features + 8 mask
KP = 100                  # DoubleRow partitions
USE_DMAT = False          # DMA (XBAR) transpose vs PE transpose for aot
DEBUG_DUMP = False        # add chunk-0 intermediate dumps as outputs

_CACHED = {}


def _fp8(a):
    import ml_dtypes
    return a.astype(ml_dtypes.float8_e4m3fn)


def _f32(a):
    return np.asarray(a, np.float32)


def _pack_dr(a):
    """[KL, M] -> DoubleRow fold-100 layout [KP, 2*M]: (p, s*M+m) = a[s*KP+p, m]."""
    m = a.shape[1]
    return np.ascontiguousarray(
        a.reshape(2, KP, m).transpose(1, 0, 2).reshape(KP, 2 * m))


def _build_host_constants(W_qkv, b_qkv, key_rel, key_rel_diag, W_out, b_out):
    f16 = np.float16
    scale = np.float32(SCALE)

    # QK projection weights, q columns pre-scaled; slab order
    # slabA = [q^h0; q^h1], slabB = [k_h0; k_h1], slabC = [q^h2; k_h2].
    qs = W_qkv[:, 0:DIM] * (scale * SQ)
    kk = W_qkv[:, DIM:2 * DIM] * SW
    wqk = np.concatenate(
        [qs[:, 0:128], kk[:, 0:128], qs[:, 128:192], kk[:, 128:192]], axis=1)
    wv = W_qkv[:, 2 * DIM:3 * DIM] * SW

    # KRABS[i, j] = relative key vector seen by query position i at absolute
    # key position j (diag vector on j == i).
    kr = key_rel.reshape(V, V - 1, DK)
    KRABS = np.zeros((V, V, DK), np.float32)
    for i in range(V):
        for j in range(V):
            KRABS[i, j] = key_rel_diag[0] if j == i else kr[i, j - (j > i)]

    # wrel[i]: (192, 96). Columns 32h + j' (j' < 17) hold
    # SREL * scale * W_q[:, head h] @ KRABS[i, j'].
    wrel = np.zeros((V, DIM, 96), np.float32)
    for h in range(H):
        wq_h = W_qkv[:, h * DK:(h + 1) * DK]          # (192, 64)
        proj = np.einsum('dk,ijk->dij', wq_h, KRABS) * (scale * SREL)
        for i in range(V):
            wrel[i, :, 32 * h:32 * h + 17] = proj[:, i, :]

    # Static patterns over a chunk's 2176 tokens.
    t = np.arange(TC)
    pos = t % V               # position within sequence
    grp = (t // V) % 7        # batch index within 119-token group
    # eml: one-hot key rows (ONEHOT = 2048/SREL) + mask rows.
    eml = np.zeros((26, TC), np.float32)
    for j in range(V):
        eml[j] = ONEHOT * (pos == j)
    for a in range(7):
        eml[17 + a] = C2 * (grp == a)
    eml[24] = 0.0
    eml[25] = C4
    emlp = np.concatenate([eml, np.zeros((6, TC), np.float32)], axis=0)
    eml3 = np.concatenate([emlp, emlp, emlp], axis=0)   # (96, TC)

    # mgrp: static rows 192:200 of the fp8 x tensor (hi only).
    mgrp = np.zeros((8, TC), np.float32)
    for a in range(7):
        mgrp[a] = (grp == a)
    mgrp[7] = 1.0

    # wrel extended to KL rows: rows 192:200 hold the mask-generation
    # constants (exact in fp8): row 192+a pairs with mgrp[a] to emit C1 into
    # frm's mask rows; row 199 pairs with the ones row to emit C3.
    wrel_ext = np.zeros((KL, V * 96), np.float32)
    wrel_ext[0:DIM] = wrel.transpose(1, 0, 2).reshape(DIM, V * 96)
    for h in range(H):
        for a in range(7):
            for i in range(V):
                wrel_ext[192 + a, i * 96 + 32 * h + 17 + a] = C1
        for i in range(V):
            wrel_ext[199, i * 96 + 32 * h + 25] = C3

    wqk_ext = np.zeros((KL, 384), np.float32)
    wqk_ext[0:DIM] = wqk
    wv_ext = np.zeros((KL, 192), np.float32)
    wv_ext[0:DIM] = wv

    def dr_variants(ext):
        hi = _f32(_fp8(ext))
        lo = _f32(_fp8(ext - hi))
        hi16 = _f32(_fp8(hi / 16.0))
        return (_pack_dr(_fp8(hi)), _pack_dr(_fp8(hi16)), _pack_dr(_fp8(lo)))

    qk_hi, qk_hi16, qk_lo = dr_variants(wqk_ext)
    v_hi, v_hi16, v_lo = dr_variants(wv_ext)
    r_hi, r_hi16, r_lo = dr_variants(wrel_ext)

    consts = {
        "wqk_hi": qk_hi, "wqk_hi16": qk_hi16, "wqk_lo": qk_lo,
        "wv_hi": v_hi, "wv_hi16": v_hi16, "wv_lo": v_lo,
        "wrel_hi": r_hi, "wrel_hi16": r_hi16, "wrel_lo": r_lo,
        "wout0": (W_out[0:128] / SW).astype(f16),
        "wout1": (W_out[128:192] / SW).astype(f16),
        "eml": eml3.astype(f16),
        "ident": np.eye(128, dtype=f16),
    }
    return consts, mgrp


def _build_bass():
    import concourse.bacc as bacc
    import concourse.mybir as mybir
    from concourse import tile

    f16 = mybir.dt.float16
    f32 = mybir.dt.float32
    f8 = mybir.dt.float8e4
    EXP = mybir.ActivationFunctionType.Exp
    MUL = mybir.AluOpType.mult
    DR = mybir.MatmulPerfMode.DoubleRow

    nc = bacc.Bacc(None, target_bir_lowering=False)

    def dp(name, shape, dt=f16):
        return nc.declare_dram_parameter(name, list(shape), dt, isOutput=False)

    x_hi_in = dp("x_hi", (KP, 2 * TOK), f8)
    x_lo_in = dp("x_lo", (KP, 2 * TOK), f8)
    wqk_d = [dp(f"wqk_{s}", (KP, 2 * 384), f8) for s in ("hi", "hi16", "lo")]
    wv_d = [dp(f"wv_{s}", (KP, 2 * 192), f8) for s in ("hi", "hi16", "lo")]
    wrel_d = [dp(f"wrel_{s}", (KP, 2 * V * 96), f8) for s in ("hi", "hi16", "lo")]
    wout0_d = dp("wout0", (128, 192))
    wout1_d = dp("wout1", (64, 192))
    eml_d = dp("eml", (96, TC))
    ident_d = dp("ident", (128, 128))
    y_out = nc.declare_dram_parameter("y", [TOK, DIM], f32, isOutput=True)
    dbg = {}
    if DEBUG_DUMP:
        for nm, shp, dt in [("d_qka", (128, TC), f16), ("d_qkb", (128, TC), f16),
                            ("d_qkc", (128, TC), f16), ("d_frm", (96, TC), f16),
                            ("d_attn", (119, NGH * 119), f16),
                            ("d_vt", (119, G * 195), f16),
                            ("d_avout", (119, G * 192), f16),
                            ("d_aotA", (128, TC), f16), ("d_aotB", (128, TC), f16)]:
            dbg[nm] = nc.declare_dram_parameter(nm, list(shp), dt, isOutput=True)

    NT512 = [(0, 512), (512, 512), (1024, 512), (1536, 512), (2048, 128)]

    from contextlib import ExitStack
    with tile.TileContext(nc) as tc, ExitStack() as es:
        wp = es.enter_context(tc.sbuf_pool(name="wpool", bufs=1))
        sp = es.enter_context(tc.sbuf_pool(name="work", bufs=2))
        dpool = es.enter_context(tc.tile_pool(name="dsc", space="DRAM", bufs=2))
        psE = es.enter_context(tc.psum_pool(name="psE", bufs=3))
        psL = es.enter_context(tc.psum_pool(name="psL", bufs=3))
        pst = None if USE_DMAT else es.enter_context(tc.psum_pool(name="pst", bufs=1))
        if True:

            # ---- persistent weights ----
            wqk8 = []
            for s, d in zip(("hi", "hi16", "lo"), wqk_d):
                t8 = wp.tile([KP, 2 * 384], f8, name=f"wqk8_{s}")
                nc.sync.dma_start(out=t8[:], in_=d[:])
                wqk8.append(t8[:].rearrange("p (s m) -> p s m", s=2))
            wv8 = []
            for s, d in zip(("hi", "hi16", "lo"), wv_d):
                t8 = wp.tile([KP, 2 * 192], f8, name=f"wv8_{s}")
                nc.sync.dma_start(out=t8[:], in_=d[:])
                wv8.append(t8[:].rearrange("p (s m) -> p s m", s=2))
            wrel8 = []
            for s, d in zip(("hi", "hi16", "lo"), wrel_d):
                t8 = wp.tile([KP, 2 * V * 96], f8, name=f"wrel8_{s}")
                nc.sync.dma_start(out=t8[:], in_=d[:])
                wrel8.append(t8[:].rearrange("p (s i m) -> p s i m", s=2, m=96))
            wout0 = wp.tile([128, 192], f16)
            nc.sync.dma_start(out=wout0[:], in_=wout0_d[:])
            woutB = wp.tile([128, 192], f16)
            nc.sync.dma_start(out=woutB[64:128, :], in_=wout1_d[:])
            eml = wp.tile([96, TC], f16)
            nc.sync.dma_start(out=eml[:], in_=eml_d[:])
            ident = wp.tile([128, 128], f16)
            nc.sync.dma_start(out=ident[:], in_=ident_d[:])

            prev = {}

            def emit_po(pv):
                # out-projection + store for a completed (transposed) chunk
                aotA, aotB, r0p = pv["aotA"], pv["aotB"], pv["r0"]
                fin = sp.tile([128, 17 * 192], f32, tag="fin")
                finv = fin[:].rearrange("p (t c) -> p t c", c=192)
                for tp in range(9):          # packs of 2 token-tiles
                    npo = min(2, 17 - tp * 2)
                    po = psL.tile([128, 512], f32, tag="psL")
                    for u in range(npo):
                        t = tp * 2 + u
                        gc = slice(t * 128, t * 128 + 128)
                        nc.tensor.matmul(po[:, u * 256:u * 256 + 192],
                                         aotA[:, gc], wout0[:],
                                         start=True, stop=False)
                        nc.tensor.matmul(po[:, u * 256:u * 256 + 192],
                                         aotB[64:128, gc], woutB[64:128, :],
                                         start=False, stop=True)
                    dst = finv[:, tp * 2:tp * 2 + npo, :]
                    src_ = po[:, 0:npo * 256].rearrange(
                        "p (u c) -> p u c", c=256)[:, :, 0:192]
                    if tp % 2 == 0:
                        nc.scalar.copy(dst, src_)
                    else:
                        nc.vector.tensor_copy(dst, src_)
                for s0, s1 in ((0, 6), (6, 12), (12, 17)):
                    nc.scalar.dma_start(
                        out=y_out[r0p + s0 * 128:r0p + s1 * 128, :].rearrange(
                            "(t p) d -> p t d", p=128),
                        in_=fin[:, s0 * 192:s1 * 192].rearrange(
                            "p (t d) -> p t d", d=192))

            for c in range(NCHUNK):
                r0 = c * TC
                # ---- fp8 x loads (DoubleRow fold-100 layout) ----
                xhi = sp.tile([KP, 2 * TC], f8, tag="xhi")
                xlo = sp.tile([KP, 2 * TC], f8, tag="xlo")
                nc.gpsimd.dma_start(
                    out=xhi[:].rearrange("p (s t) -> p s t", s=2),
                    in_=x_hi_in[:].rearrange("p (s t) -> p s t", s=2)[:, :, r0:r0 + TC])
                nc.gpsimd.dma_start(
                    out=xlo[:].rearrange("p (s t) -> p s t", s=2),
                    in_=x_lo_in[:].rearrange("p (s t) -> p s t", s=2)[:, :, r0:r0 + TC])
                if prev:
                    emit_po(prev)
                xhiv = xhi[:].rearrange("p (s t) -> p s t", s=2)
                xlov = xlo[:].rearrange("p (s t) -> p s t", s=2)
                xhip = xhi[:].rearrange("p (s b v) -> p s b v", s=2, v=V)
                xlop = xlo[:].rearrange("p (s b v) -> p s b v", s=2, v=V)

                # ---- rel projections -> frm (96, TC), packs of 8 positions ----
                frm = sp.tile([96, TC], f16, tag="frm")
                frmv = frm[:].rearrange("p (b v) -> p b v", v=V)
                for ip in range(5):          # packs of 4 positions
                    n = min(4, V - ip * 4)
                    pr = psE.tile([128, 512], f32, tag="psE")
                    for u in range(n):
                        i = ip * 4 + u
                        o = u * 128
                        nc.tensor.matmul(pr[0:96, o:o + 128],
                                         wrel8[0][:, :, i, :], xhip[:, :, :, i],
                                         start=True, stop=False, perf_mode=DR)
                        nc.tensor.matmul(pr[0:96, o:o + 128],
                                         wrel8[1][:, :, i, :], xlop[:, :, :, i],
                                         start=False, stop=False, perf_mode=DR)
                        nc.tensor.matmul(pr[0:96, o:o + 128],
                                         wrel8[2][:, :, i, :], xhip[:, :, :, i],
                                         start=False, stop=True, perf_mode=DR)
                    nc.vector.tensor_copy(
                        frmv[:, :, ip * 4:ip * 4 + n],
                        pr[0:96, 0:n * 128].rearrange("p (i b) -> p b i", b=128))

                # ---- QK^T projections -> 3 slabs ----
                qka = sp.tile([128, TC], f16, tag="qka")
                qkb = sp.tile([128, TC], f16, tag="qkb")
                qkc = sp.tile([128, TC], f16, tag="qkc")
                slabs = [qka, qkb, qkc]
                kh2t = sp.tile([64, TC], f16, tag="kh2t")
                ei = 0
                for m in (2, 0, 1):
                    for n0, nw in NT512:
                        pq = psE.tile([128, 512], f32, tag="psE")
                        nc.tensor.matmul(pq[:, 0:nw],
                                         wqk8[0][:, :, m * 128:(m + 1) * 128],
                                         xhiv[:, :, n0:n0 + nw],
                                         start=True, stop=False, perf_mode=DR)
                        nc.tensor.matmul(pq[:, 0:nw],
                                         wqk8[1][:, :, m * 128:(m + 1) * 128],
                                         xlov[:, :, n0:n0 + nw],
                                         start=False, stop=False, perf_mode=DR)
                        nc.tensor.matmul(pq[:, 0:nw],
                                         wqk8[2][:, :, m * 128:(m + 1) * 128],
                                         xhiv[:, :, n0:n0 + nw],
                                         start=False, stop=True, perf_mode=DR)
                        if ei % 2 == 0:
                            nc.scalar.copy(slabs[m][:, n0:n0 + nw], pq[:, 0:nw])
                        else:
                            nc.vector.tensor_copy(slabs[m][:, n0:n0 + nw],
                                                  pq[:, 0:nw])
                        ei += 1
                        if m == 2:
                            nc.vector.tensor_copy(kh2t[:, n0:n0 + nw],
                                                  qkc[64:128, n0:n0 + nw])

                # ---- dots^T + rel + mask, exp; packs of 8 (g,h) tiles ----
                QT = [qka[0:64, :], qka[64:128, :], qkc[0:64, :]]
                KT = [qkb[0:64, :], qkb[64:128, :], kh2t[0:64, :]]
                attn = sp.tile([119, NGH * 119], f16, tag="attn")
                for pk in range(15):         # packs of 4 (g,h) tiles
                    n = min(4, NGH - pk * 4)
                    pd = psE.tile([128, 512], f32, tag="psE")
                    for u in range(n):
                        idx = pk * 4 + u
                        g, h = divmod(idx, H)
                        gs = GSIZES[g]
                        gc = slice(GOFFS[g], GOFFS[g] + gs)
                        o = u * 128
                        nc.tensor.matmul(pd[0:gs, o:o + gs], KT[h][:, gc],
                                         QT[h][:, gc], start=True, stop=False)
                        nc.tensor.matmul(pd[0:gs, o:o + gs],
                                         eml[32 * h:32 * h + 26, gc],
                                         frm[32 * h:32 * h + 26, gc],
                                         start=False, stop=True)
                    pr_rows = 119 if n > 1 else GSIZES[-1]
                    nc.scalar.activation(
                        attn[0:pr_rows, pk * 476:pk * 476 + n * 119].rearrange(
                            "p (u c) -> p u c", c=119),
                        pd[0:pr_rows, 0:n * 128].rearrange(
                            "p (u c) -> p u c", c=128)[:, :, 0:119],
                        EXP, scale=EXPSCALE)

                # ---- V projection (token-major, +ones column) ----
                vt = sp.tile([119, G * 195], f16, tag="vt")
                nc.gpsimd.memset(
                    vt[:].rearrange("p (g hh c) -> p g hh c", hh=3, c=65)[:, :, :, 64:65],
                    1.0)
                vtv = vt[:].rearrange("p (g hh c) -> p g hh c", hh=3, c=65)
                for gp in range(10):         # packs of 2 groups
                    n = min(2, G - gp * 2)
                    pv = psL.tile([128, 512], f32, tag="psL")
                    for u in range(n):
                        g = gp * 2 + u
                        gs = GSIZES[g]
                        gc = slice(GOFFS[g], GOFFS[g] + gs)
                        nc.tensor.matmul(pv[0:gs, u * 256:u * 256 + 192],
                                         xhiv[:, :, gc], wv8[0],
                                         start=True, stop=False, perf_mode=DR)
                        nc.tensor.matmul(pv[0:gs, u * 256:u * 256 + 192],
                                         xlov[:, :, gc], wv8[1],
                                         start=False, stop=False, perf_mode=DR)
                        nc.tensor.matmul(pv[0:gs, u * 256:u * 256 + 192],
                                         xhiv[:, :, gc], wv8[2],
                                         start=False, stop=True, perf_mode=DR)
                    g0 = gp * 2
                    vr = 119 if n > 1 else GSIZES[-1]
                    nc.vector.tensor_copy(
                        vtv[0:vr, g0:g0 + n, :, 0:64],
                        pv[0:vr, 0:n * 256].rearrange(
                            "p (u hh c) -> p u hh c", hh=4, c=64)[:, :, 0:3, :])

                # ---- attention @ V (+denominator), normalize on eviction ----
                avout = sp.tile([119, G * 192], f16, tag="avout")
                avv = avout[:].rearrange("p (g hh c) -> p g hh c", hh=3, c=64)
                recip = sp.tile([119, NGH], f32, tag="recip")
                recv = recip[:].rearrange("p (g hh) -> p g hh", hh=3)
                for gp in range(10):         # packs of 2 groups
                    n = min(2, G - gp * 2)
                    pa = psL.tile([128, 512], f32, tag="psL")
                    for u in range(n):
                        g = gp * 2 + u
                        gs = GSIZES[g]
                        for h in range(H):
                            idx = g * H + h
                            nc.tensor.matmul(
                                pa[0:gs, u * 256 + 65 * h:u * 256 + 65 * h + 65],
                                attn[0:gs, idx * 119:idx * 119 + gs],
                                vtv[0:gs, g, h, :],
                                start=True, stop=True)
                    g0 = gp * 2
                    ar = 119 if n > 1 else GSIZES[-1]
                    pav = pa[0:ar, 0:n * 256].rearrange(
                        "p (u q) -> p u q", q=256)[:, :, 0:195].rearrange(
                        "p u (hh c) -> p u hh c", c=65)
                    nc.vector.reciprocal(recv[0:ar, g0:g0 + n, :], pav[:, :, :, 64])
                    nc.vector.tensor_tensor(
                        avv[0:ar, g0:g0 + n, :, :],
                        pav[:, :, :, 0:64],
                        recv[0:ar, g0:g0 + n, :].broadcast_to([ar, n, 3, 64]),
                        MUL)

                # ---- feature-major via DMA transpose (XBAR) ----
                aotA = sp.tile([128, TC], f16, tag="aotA")
                aotB = sp.tile([128, TC], f16, tag="aotB")
                if USE_DMAT:
                    sc = dpool.tile([TC, 192], f16, tag="dsc")
                    nc.sync.dma_start(
                        out=sc[0:10 * 119, :].rearrange("(g p) d -> p g d", p=119),
                        in_=avout[0:119, 0:10 * 192].rearrange("p (g d) -> p g d", d=192))
                    nc.sync.dma_start(
                        out=sc[10 * 119:18 * 119, :].rearrange("(g p) d -> p g d", p=119),
                        in_=avout[0:119, 10 * 192:18 * 192].rearrange("p (g d) -> p g d", d=192))
                    nc.sync.dma_start(out=sc[18 * 119:TC, :],
                                      in_=avout[0:34, 18 * 192:19 * 192])
                    nc.sync.dma_start_transpose(aotA[:], sc[:, 0:128])
                    nc.sync.dma_start_transpose(aotB[:], sc[:, 64:192])
                else:
                    # PE transposes (baseline path): avout [119, g, 192] ->
                    # aotA feats 0:128, aotB rows 64:128 = feats 128:192.
                    for gp in range(5):          # packs of 4 groups
                        n = min(4, G - gp * 4)
                        pc = pst.tile([128, 512], f16, tag="pst")
                        pe = pst.tile([128, 512], f16, tag="pst")
                        for u in range(n):
                            g = gp * 4 + u
                            gs = GSIZES[g]
                            nc.tensor.transpose(pc[:, u * 128:u * 128 + gs],
                                                avout[0:gs, g * 192:g * 192 + 128],
                                                ident[0:gs, 0:gs])
                            nc.tensor.transpose(pe[0:64, u * 128:u * 128 + gs],
                                                avout[0:gs, g * 192 + 128:g * 192 + 192],
                                                ident[0:gs, 0:gs])
                        t0 = GOFFS[gp * 4]
                        if n == 4 and GSIZES[gp * 4 + 3] == 119:
                            nc.vector.tensor_copy(
                                aotA[:, t0:t0 + 476].rearrange("p (u c) -> p u c", c=119),
                                pc[:, :].rearrange("p (u c) -> p u c", c=128)[:, 0:4, 0:119])
                            nc.vector.tensor_copy(
                                aotB[64:128, t0:t0 + 476].rearrange("p (u c) -> p u c", c=119),
                                pe[0:64, :].rearrange("p (u c) -> p u c", c=128)[:, 0:4, 0:119])
                        else:
                            for u in range(n):
                                g = gp * 4 + u
                                gs = GSIZES[g]
                                gt = GOFFS[g]
                                nc.vector.tensor_copy(aotA[:, gt:gt + gs],
                                                      pc[:, u * 128:u * 128 + gs])
                                nc.vector.tensor_copy(aotB[64:128, gt:gt + gs],
                                                      pe[0:64, u * 128:u * 128 + gs])

                prev = {"aotA": aotA, "aotB": aotB, "r0": r0}

            emit_po(prev)

    nc.finalize()
    return nc


def kernel(x, W_qkv, b_qkv, key_rel, key_rel_diag, W_out, b_out):
    from concourse.bass_utils import run_bass_kernel_spmd

    x = np.ascontiguousarray(np.asarray(x, dtype=np.float32))
    consts, mgrp = _build_host_constants(
        _f32(W_qkv), _f32(b_qkv), _f32(key_rel), _f32(key_rel_diag),
        _f32(W_out), _f32(b_out))

    if "nc" not in _CACHED:
        _CACHED["nc"] = _build_bass()
    nc = _CACHED["nc"]

    # x_ext: [NCORES, KL, TOK] logical rows = 192 features + 8 mask rows,
    # then hi/lo split and DoubleRow fold-100 packing.
    xT = x.reshape(NCORES, BC * V, DIM).transpose(0, 2, 1)  # [NC, 192, TOK]
    mgrp_full = np.tile(mgrp, (1, NCHUNK))                  # [8, TOK]
    x_hi_l = []
    x_lo_l = []
    for k in range(NCORES):
        ext = np.concatenate([xT[k], mgrp_full], axis=0)    # [200, TOK]
        hi = _f32(_fp8(ext))
        lo = (ext - hi) * 16.0
        lo[DIM:KL] = 0.0
        x_hi_l.append(_pack_dr(_fp8(hi)))
        x_lo_l.append(_pack_dr(_fp8(lo)))

    in_maps = [dict(consts, x_hi=x_hi_l[k], x_lo=x_lo_l[k])
               for k in range(NCORES)]
    res = run_bass_kernel_spmd(nc, in_maps, core_ids=list(range(NCORES)))
    _CACHED["last_result"] = res
    out = np.stack([res.results[k]["y"] for k in range(NCORES)], axis=0)
    out = out.reshape(B, V, DIM)
    # Value bias passes through normalized attention as a constant; fold it
    # (and b_out) exactly on the host. The q/k biases are zeros per the spec.
    bv = _f32(b_qkv)[2 * DIM:3 * DIM]
    bias = bv @ _f32(W_out) + _f32(b_out)
    if np.any(bias):
        out = out + bias
    return out


# revision 27
# speedup vs baseline: 1.1063x; 1.0002x over previous
"""Trainium2 Bass kernel for nn_Attention_6133213298828.

Batch-parallel multi-head attention with per-query-position relative-position
logits, forward pass only. Data-parallel over 8 NeuronCores (batch dim);
weights replicated, no collectives.

Per-core design (2048 batches, 16 chunks of 128 batches / 2176 tokens):
  - All three x-consuming projections (QK, rel, V) run as compensated fp8e4
    DoubleRow matmuls: 3 half-rate matmuls (W_hi*x_hi + (W_hi/16)*(16*x_lo)
    + W_lo*x_hi) replace 2 full-rate fp16 matmuls, ~25% less PE streaming at
    ~0.15% rel error. x ships as fp8 hi + fp8(16*lo) pairs in the DoubleRow
    fold-100 layout (K=200 logical rows: 192 features + 8 static mask rows).
    Weights are scaled into fp8 normal range (q: 64x, k/v: 32x, wrel: 512x);
    the scale cancels via exp(score * 2^-11) and W_out/32. All mask/one-hot
    constants are chosen exactly representable in fp8/f16 (and <= 448, the
    e4m3 max: the -480 ones-row constant of an earlier revision became fp8
    NaN and poisoned everything).
  - Attention on 119-token groups (7 batches x 17 positions <= 128
    partitions); relative logits fold into the score matmul as 26 extra
    contraction rows (one-hot key rows against per-token rel projections);
    cross-batch garbage killed by an additive -30*2048 mask generated by
    static indicator rows riding the fp8 x tensor.
  - Softmax denominators via a ones column in V; normalization multiplies
    the AV PSUM rows by a broadcast reciprocal during eviction; AV output is
    PE-transposed back to feature-major for the output projection.
  - Engine balance (the kernel is eviction-bound on Act+DVE, not PE-bound):
    Act runs exp + most QK/out-proj evictions; DVE runs frm/V/normalize
    evictions, transposes eviction, and 2 of the 15 QK eviction windows;
    the k_h2 re-basing copy runs on the otherwise-idle Pool (gpsimd) engine
    (SBUF->SBUF only - gpsimd cannot touch PSUM); input DMA on the gpsimd
    SWDGE queue, output DMA on sync/HWDGE, avoiding head-of-line blocking
    between loads and stores on one queue.
"""

import numpy as np

DIM, OUT_DIM, H, V, B = 192, 192, 3, 17, 16384
DK = DIM // H
NCORES = 8
BC = B // NCORES          # batches per core
NB = 128                  # batches per chunk
NCHUNK = BC // NB         # 16
TC = NB * V               # 2176 tokens per chunk
TOK = BC * V              # 34816 tokens per core
GSIZES = [119] * 18 + [34]            # token-group sizes within a chunk
GOFFS = np.cumsum([0] + GSIZES).tolist()
G = len(GSIZES)           # 19
NGH = G * H               # 57 (group, head) tiles per chunk
SCALE = DIM ** -0.5
SW = 32.0                 # fp8 weight scale (k/v)
SQ = 64.0                 # fp8 weight scale (q); scores at 2048x
SREL = 512.0              # fp8 wrel scale
ONEHOT = 4.0              # eml one-hot value = 2048/SREL
C1 = 192.0                # frm mask-row magnitude (exact fp8, <= 448)
C2 = 320.0                # eml mask-row magnitude; C1*C2 = 61440 = 30*2048
C3 = -240.0               # frm ones-row value (exact fp8, <= 448)
C4 = 256.0                # eml ones-row value; C3*C4 = -61440
EXPSCALE = float(2.0 ** -11)

KL = 200                  # logical contraction rows: 192 features + 8 mask
KP = 100                  # DoubleRow partitions
USE_DMAT = False          # DMA (XBAR) transpose vs PE transpose for aot
DEBUG_DUMP = False        # add chunk-0 intermediate dumps as outputs

_CACHED = {}


def _fp8(a):
    import ml_dtypes
    return a.astype(ml_dtypes.float8_e4m3fn)


def _f32(a):
    return np.asarray(a, np.float32)


def _pack_dr(a):
    """[KL, M] -> DoubleRow fold-100 layout [KP, 2*M]: (p, s*M+m) = a[s*KP+p, m]."""
    m = a.shape[1]
    return np.ascontiguousarray(
        a.reshape(2, KP, m).transpose(1, 0, 2).reshape(KP, 2 * m))


def _build_host_constants(W_qkv, b_qkv, key_rel, key_rel_diag, W_out, b_out):
    f16 = np.float16
    scale = np.float32(SCALE)

    # QK projection weights, q columns pre-scaled; slab order
    # slabA = [q^h0; q^h1], slabB = [k_h0; k_h1], slabC = [q^h2; k_h2].
    qs = W_qkv[:, 0:DIM] * (scale * SQ)
    kk = W_qkv[:, DIM:2 * DIM] * SW
    wqk = np.concatenate(
        [qs[:, 0:128], kk[:, 0:128], qs[:, 128:192], kk[:, 128:192]], axis=1)
    wv = W_qkv[:, 2 * DIM:3 * DIM] * SW

    # KRABS[i, j] = relative key vector seen by query position i at absolute
    # key position j (diag vector on j == i).
    kr = key_rel.reshape(V, V - 1, DK)
    KRABS = np.zeros((V, V, DK), np.float32)
    for i in range(V):
        for j in range(V):
            KRABS[i, j] = key_rel_diag[0] if j == i else kr[i, j - (j > i)]

    # wrel[i]: (192, 96). Columns 32h + j' (j' < 17) hold
    # SREL * scale * W_q[:, head h] @ KRABS[i, j'].
    wrel = np.zeros((V, DIM, 96), np.float32)
    for h in range(H):
        wq_h = W_qkv[:, h * DK:(h + 1) * DK]          # (192, 64)
        proj = np.einsum('dk,ijk->dij', wq_h, KRABS) * (scale * SREL)
        for i in range(V):
            wrel[i, :, 32 * h:32 * h + 17] = proj[:, i, :]

    # Static patterns over a chunk's 2176 tokens.
    t = np.arange(TC)
    pos = t % V               # position within sequence
    grp = (t // V) % 7        # batch index within 119-token group
    # eml: one-hot key rows (ONEHOT = 2048/SREL) + mask rows.
    eml = np.zeros((26, TC), np.float32)
    for j in range(V):
        eml[j] = ONEHOT * (pos == j)
    for a in range(7):
        eml[17 + a] = C2 * (grp == a)
    eml[24] = 0.0
    eml[25] = C4
    emlp = np.concatenate([eml, np.zeros((6, TC), np.float32)], axis=0)
    eml3 = np.concatenate([emlp, emlp, emlp], axis=0)   # (96, TC)

    # mgrp: static rows 192:200 of the fp8 x tensor (hi only).
    mgrp = np.zeros((8, TC), np.float32)
    for a in range(7):
        mgrp[a] = (grp == a)
    mgrp[7] = 1.0

    # wrel extended to KL rows: rows 192:200 hold the mask-generation
    # constants (exact in fp8): row 192+a pairs with mgrp[a] to emit C1 into
    # frm's mask rows; row 199 pairs with the ones row to emit C3.
    wrel_ext = np.zeros((KL, V * 96), np.float32)
    wrel_ext[0:DIM] = wrel.transpose(1, 0, 2).reshape(DIM, V * 96)
    for h in range(H):
        for a in range(7):
            for i in range(V):
                wrel_ext[192 + a, i * 96 + 32 * h + 17 + a] = C1
        for i in range(V):
            wrel_ext[199, i * 96 + 32 * h + 25] = C3

    wqk_ext = np.zeros((KL, 384), np.float32)
    wqk_ext[0:DIM] = wqk
    wv_ext = np.zeros((KL, 192), np.float32)
    wv_ext[0:DIM] = wv

    def dr_variants(ext):
        hi = _f32(_fp8(ext))
        lo = _f32(_fp8(ext - hi))
        hi16 = _f32(_fp8(hi / 16.0))
        return (_pack_dr(_fp8(hi)), _pack_dr(_fp8(hi16)), _pack_dr(_fp8(lo)))

    qk_hi, qk_hi16, qk_lo = dr_variants(wqk_ext)
    v_hi, v_hi16, v_lo = dr_variants(wv_ext)
    r_hi, r_hi16, r_lo = dr_variants(wrel_ext)

    consts = {
        "wqk_hi": qk_hi, "wqk_hi16": qk_hi16, "wqk_lo": qk_lo,
        "wv_hi": v_hi, "wv_hi16": v_hi16, "wv_lo": v_lo,
        "wrel_hi": r_hi, "wrel_hi16": r_hi16, "wrel_lo": r_lo,
        "wout0": (W_out[0:128] / SW).astype(f16),
        "wout1": (W_out[128:192] / SW).astype(f16),
        "eml": eml3.astype(f16),
        "ident": np.eye(128, dtype=f16),
    }
    return consts, mgrp


def _build_bass():
    import concourse.bacc as bacc
    import concourse.mybir as mybir
    from concourse import tile

    f16 = mybir.dt.float16
    f32 = mybir.dt.float32
    f8 = mybir.dt.float8e4
    EXP = mybir.ActivationFunctionType.Exp
    MUL = mybir.AluOpType.mult
    DR = mybir.MatmulPerfMode.DoubleRow

    nc = bacc.Bacc(None, target_bir_lowering=False)

    def dp(name, shape, dt=f16):
        return nc.declare_dram_parameter(name, list(shape), dt, isOutput=False)

    x_hi_in = dp("x_hi", (KP, 2 * TOK), f8)
    x_lo_in = dp("x_lo", (KP, 2 * TOK), f8)
    wqk_d = [dp(f"wqk_{s}", (KP, 2 * 384), f8) for s in ("hi", "hi16", "lo")]
    wv_d = [dp(f"wv_{s}", (KP, 2 * 192), f8) for s in ("hi", "hi16", "lo")]
    wrel_d = [dp(f"wrel_{s}", (KP, 2 * V * 96), f8) for s in ("hi", "hi16", "lo")]
    wout0_d = dp("wout0", (128, 192))
    wout1_d = dp("wout1", (64, 192))
    eml_d = dp("eml", (96, TC))
    ident_d = dp("ident", (128, 128))
    y_out = nc.declare_dram_parameter("y", [TOK, DIM], f32, isOutput=True)
    dbg = {}
    if DEBUG_DUMP:
        for nm, shp, dt in [("d_qka", (128, TC), f16), ("d_qkb", (128, TC), f16),
                            ("d_qkc", (128, TC), f16), ("d_frm", (96, TC), f16),
                            ("d_attn", (119, NGH * 119), f16),
                            ("d_vt", (119, G * 195), f16),
                            ("d_avout", (119, G * 192), f16),
                            ("d_aotA", (128, TC), f16), ("d_aotB", (128, TC), f16)]:
            dbg[nm] = nc.declare_dram_parameter(nm, list(shp), dt, isOutput=True)

    NT512 = [(0, 512), (512, 512), (1024, 512), (1536, 512), (2048, 128)]

    from contextlib import ExitStack
    with tile.TileContext(nc) as tc, ExitStack() as es:
        wp = es.enter_context(tc.sbuf_pool(name="wpool", bufs=1))
        sp = es.enter_context(tc.sbuf_pool(name="work", bufs=2))
        dpool = es.enter_context(tc.tile_pool(name="dsc", space="DRAM", bufs=2))
        psE = es.enter_context(tc.psum_pool(name="psE", bufs=3))
        psL = es.enter_context(tc.psum_pool(name="psL", bufs=3))
        pst = None if USE_DMAT else es.enter_context(tc.psum_pool(name="pst", bufs=1))
        if True:

            # ---- persistent weights ----
            wqk8 = []
            for s, d in zip(("hi", "hi16", "lo"), wqk_d):
                t8 = wp.tile([KP, 2 * 384], f8, name=f"wqk8_{s}")
                nc.sync.dma_start(out=t8[:], in_=d[:])
                wqk8.append(t8[:].rearrange("p (s m) -> p s m", s=2))
            wv8 = []
            for s, d in zip(("hi", "hi16", "lo"), wv_d):
                t8 = wp.tile([KP, 2 * 192], f8, name=f"wv8_{s}")
                nc.sync.dma_start(out=t8[:], in_=d[:])
                wv8.append(t8[:].rearrange("p (s m) -> p s m", s=2))
            wrel8 = []
            for s, d in zip(("hi", "hi16", "lo"), wrel_d):
                t8 = wp.tile([KP, 2 * V * 96], f8, name=f"wrel8_{s}")
                nc.sync.dma_start(out=t8[:], in_=d[:])
                wrel8.append(t8[:].rearrange("p (s i m) -> p s i m", s=2, m=96))
            wout0 = wp.tile([128, 192], f16)
            nc.sync.dma_start(out=wout0[:], in_=wout0_d[:])
            woutB = wp.tile([128, 192], f16)
            nc.sync.dma_start(out=woutB[64:128, :], in_=wout1_d[:])
            eml = wp.tile([96, TC], f16)
            nc.sync.dma_start(out=eml[:], in_=eml_d[:])
            ident = wp.tile([128, 128], f16)
            nc.sync.dma_start(out=ident[:], in_=ident_d[:])

            prev = {}

            def emit_po(pv):
                # out-projection + store for a completed (transposed) chunk
                aotA, aotB, r0p = pv["aotA"], pv["aotB"], pv["r0"]
                fin = sp.tile([128, 17 * 192], f32, tag="fin")
                finv = fin[:].rearrange("p (t c) -> p t c", c=192)
                for tp in range(9):          # packs of 2 token-tiles
                    npo = min(2, 17 - tp * 2)
                    po = psL.tile([128, 512], f32, tag="psL")
                    for u in range(npo):
                        t = tp * 2 + u
                        gc = slice(t * 128, t * 128 + 128)
                        nc.tensor.matmul(po[:, u * 256:u * 256 + 192],
                                         aotA[:, gc], wout0[:],
                                         start=True, stop=False)
                        nc.tensor.matmul(po[:, u * 256:u * 256 + 192],
                                         aotB[64:128, gc], woutB[64:128, :],
                                         start=False, stop=True)
                    dst = finv[:, tp * 2:tp * 2 + npo, :]
                    src_ = po[:, 0:npo * 256].rearrange(
                        "p (u c) -> p u c", c=256)[:, :, 0:192]
                    if tp % 2 == 0:
                        nc.scalar.copy(dst, src_)
                    else:
                        nc.vector.tensor_copy(dst, src_)
                for s0, s1 in ((0, 6), (6, 12), (12, 17)):
                    nc.scalar.dma_start(
                        out=y_out[r0p + s0 * 128:r0p + s1 * 128, :].rearrange(
                            "(t p) d -> p t d", p=128),
                        in_=fin[:, s0 * 192:s1 * 192].rearrange(
                            "p (t d) -> p t d", d=192))

            for c in range(NCHUNK):
                r0 = c * TC
                # ---- fp8 x loads (DoubleRow fold-100 layout) ----
                xhi = sp.tile([KP, 2 * TC], f8, tag="xhi")
                xlo = sp.tile([KP, 2 * TC], f8, tag="xlo")
                nc.gpsimd.dma_start(
                    out=xhi[:].rearrange("p (s t) -> p s t", s=2),
                    in_=x_hi_in[:].rearrange("p (s t) -> p s t", s=2)[:, :, r0:r0 + TC])
                nc.gpsimd.dma_start(
                    out=xlo[:].rearrange("p (s t) -> p s t", s=2),
                    in_=x_lo_in[:].rearrange("p (s t) -> p s t", s=2)[:, :, r0:r0 + TC])
                if prev:
                    emit_po(prev)
                xhiv = xhi[:].rearrange("p (s t) -> p s t", s=2)
                xlov = xlo[:].rearrange("p (s t) -> p s t", s=2)
                xhip = xhi[:].rearrange("p (s b v) -> p s b v", s=2, v=V)
                xlop = xlo[:].rearrange("p (s b v) -> p s b v", s=2, v=V)

                # ---- rel projections -> frm (96, TC), packs of 8 positions ----
                frm = sp.tile([96, TC], f16, tag="frm")
                frmv = frm[:].rearrange("p (b v) -> p b v", v=V)
                for ip in range(5):          # packs of 4 positions
                    n = min(4, V - ip * 4)
                    pr = psE.tile([128, 512], f32, tag="psE")
                    for u in range(n):
                        i = ip * 4 + u
                        o = u * 128
                        nc.tensor.matmul(pr[0:96, o:o + 128],
                                         wrel8[0][:, :, i, :], xhip[:, :, :, i],
                                         start=True, stop=False, perf_mode=DR)
                        nc.tensor.matmul(pr[0:96, o:o + 128],
                                         wrel8[2][:, :, i, :], xhip[:, :, :, i],
                                         start=False, stop=True, perf_mode=DR)
                    nc.vector.tensor_copy(
                        frmv[:, :, ip * 4:ip * 4 + n],
                        pr[0:96, 0:n * 128].rearrange("p (i b) -> p b i", b=128))

                # ---- QK^T projections -> 3 slabs ----
                qka = sp.tile([128, TC], f16, tag="qka")
                qkb = sp.tile([128, TC], f16, tag="qkb")
                qkc = sp.tile([128, TC], f16, tag="qkc")
                slabs = [qka, qkb, qkc]
                kh2t = sp.tile([64, TC], f16, tag="kh2t")
                ei = 0
                for m in (2, 0, 1):
                    for n0, nw in NT512:
                        pq = psE.tile([128, 512], f32, tag="psE")
                        nc.tensor.matmul(pq[:, 0:nw],
                                         wqk8[0][:, :, m * 128:(m + 1) * 128],
                                         xhiv[:, :, n0:n0 + nw],
                                         start=True, stop=False, perf_mode=DR)
                        nc.tensor.matmul(pq[:, 0:nw],
                                         wqk8[1][:, :, m * 128:(m + 1) * 128],
                                         xlov[:, :, n0:n0 + nw],
                                         start=False, stop=False, perf_mode=DR)
                        nc.tensor.matmul(pq[:, 0:nw],
                                         wqk8[2][:, :, m * 128:(m + 1) * 128],
                                         xhiv[:, :, n0:n0 + nw],
                                         start=False, stop=True, perf_mode=DR)
                        if ei % 2 == 0:
                            nc.scalar.copy(slabs[m][:, n0:n0 + nw], pq[:, 0:nw])
                        else:
                            nc.vector.tensor_copy(slabs[m][:, n0:n0 + nw],
                                                  pq[:, 0:nw])
                        ei += 1
                        if m == 2:
                            nc.vector.tensor_copy(kh2t[:, n0:n0 + nw],
                                                  qkc[64:128, n0:n0 + nw])

                # ---- dots^T + rel + mask, exp; packs of 8 (g,h) tiles ----
                QT = [qka[0:64, :], qka[64:128, :], qkc[0:64, :]]
                KT = [qkb[0:64, :], qkb[64:128, :], kh2t[0:64, :]]
                attn = sp.tile([119, NGH * 119], f16, tag="attn")
                for pk in range(15):         # packs of 4 (g,h) tiles
                    n = min(4, NGH - pk * 4)
                    pd = psE.tile([128, 512], f32, tag="psE")
                    for u in range(n):
                        idx = pk * 4 + u
                        g, h = divmod(idx, H)
                        gs = GSIZES[g]
                        gc = slice(GOFFS[g], GOFFS[g] + gs)
                        o = u * 128
                        nc.tensor.matmul(pd[0:gs, o:o + gs], KT[h][:, gc],
                                         QT[h][:, gc], start=True, stop=False)
                        nc.tensor.matmul(pd[0:gs, o:o + gs],
                                         eml[32 * h:32 * h + 26, gc],
                                         frm[32 * h:32 * h + 26, gc],
                                         start=False, stop=True)
                    pr_rows = 119 if n > 1 else GSIZES[-1]
                    nc.scalar.activation(
                        attn[0:pr_rows, pk * 476:pk * 476 + n * 119].rearrange(
                            "p (u c) -> p u c", c=119),
                        pd[0:pr_rows, 0:n * 128].rearrange(
                            "p (u c) -> p u c", c=128)[:, :, 0:119],
                        EXP, scale=EXPSCALE)

                # ---- V projection (token-major, +ones column) ----
                vt = sp.tile([119, G * 195], f16, tag="vt")
                nc.gpsimd.memset(
                    vt[:].rearrange("p (g hh c) -> p g hh c", hh=3, c=65)[:, :, :, 64:65],
                    1.0)
                vtv = vt[:].rearrange("p (g hh c) -> p g hh c", hh=3, c=65)
                for gp in range(10):         # packs of 2 groups
                    n = min(2, G - gp * 2)
                    pv = psL.tile([128, 512], f32, tag="psL")
                    for u in range(n):
                        g = gp * 2 + u
                        gs = GSIZES[g]
                        gc = slice(GOFFS[g], GOFFS[g] + gs)
                        nc.tensor.matmul(pv[0:gs, u * 256:u * 256 + 192],
                                         xhiv[:, :, gc], wv8[0],
                                         start=True, stop=False, perf_mode=DR)
                        nc.tensor.matmul(pv[0:gs, u * 256:u * 256 + 192],
                                         xlov[:, :, gc], wv8[1],
                                         start=False, stop=False, perf_mode=DR)
                        nc.tensor.matmul(pv[0:gs, u * 256:u * 256 + 192],
                                         xhiv[:, :, gc], wv8[2],
                                         start=False, stop=True, perf_mode=DR)
                    g0 = gp * 2
                    vr = 119 if n > 1 else GSIZES[-1]
                    nc.vector.tensor_copy(
                        vtv[0:vr, g0:g0 + n, :, 0:64],
                        pv[0:vr, 0:n * 256].rearrange(
                            "p (u hh c) -> p u hh c", hh=4, c=64)[:, :, 0:3, :])

                # ---- attention @ V (+denominator), normalize on eviction ----
                avout = sp.tile([119, G * 192], f16, tag="avout")
                avv = avout[:].rearrange("p (g hh c) -> p g hh c", hh=3, c=64)
                recip = sp.tile([119, NGH], f32, tag="recip")
                recv = recip[:].rearrange("p (g hh) -> p g hh", hh=3)
                for gp in range(10):         # packs of 2 groups
                    n = min(2, G - gp * 2)
                    pa = psL.tile([128, 512], f32, tag="psL")
                    for u in range(n):
                        g = gp * 2 + u
                        gs = GSIZES[g]
                        for h in range(H):
                            idx = g * H + h
                            nc.tensor.matmul(
                                pa[0:gs, u * 256 + 65 * h:u * 256 + 65 * h + 65],
                                attn[0:gs, idx * 119:idx * 119 + gs],
                                vtv[0:gs, g, h, :],
                                start=True, stop=True)
                    g0 = gp * 2
                    ar = 119 if n > 1 else GSIZES[-1]
                    pav = pa[0:ar, 0:n * 256].rearrange(
                        "p (u q) -> p u q", q=256)[:, :, 0:195].rearrange(
                        "p u (hh c) -> p u hh c", c=65)
                    nc.vector.reciprocal(recv[0:ar, g0:g0 + n, :], pav[:, :, :, 64])
                    nc.vector.tensor_tensor(
                        avv[0:ar, g0:g0 + n, :, :],
                        pav[:, :, :, 0:64],
                        recv[0:ar, g0:g0 + n, :].broadcast_to([ar, n, 3, 64]),
                        MUL)

                # ---- feature-major via DMA transpose (XBAR) ----
                aotA = sp.tile([128, TC], f16, tag="aotA")
                aotB = sp.tile([128, TC], f16, tag="aotB")
                if USE_DMAT:
                    sc = dpool.tile([TC, 192], f16, tag="dsc")
                    nc.sync.dma_start(
                        out=sc[0:10 * 119, :].rearrange("(g p) d -> p g d", p=119),
                        in_=avout[0:119, 0:10 * 192].rearrange("p (g d) -> p g d", d=192))
                    nc.sync.dma_start(
                        out=sc[10 * 119:18 * 119, :].rearrange("(g p) d -> p g d", p=119),
                        in_=avout[0:119, 10 * 192:18 * 192].rearrange("p (g d) -> p g d", d=192))
                    nc.sync.dma_start(out=sc[18 * 119:TC, :],
                                      in_=avout[0:34, 18 * 192:19 * 192])
                    nc.sync.dma_start_transpose(aotA[:], sc[:, 0:128])
                    nc.sync.dma_start_transpose(aotB[:], sc[:, 64:192])
                else:
                    # PE transposes (baseline path): avout [119, g, 192] ->
                    # aotA feats 0:128, aotB rows 64:128 = feats 128:192.
                    for gp in range(5):          # packs of 4 groups
                        n = min(4, G - gp * 4)
                        pc = pst.tile([128, 512], f16, tag="pst")
                        pe = pst.tile([128, 512], f16, tag="pst")
                        for u in range(n):
                            g = gp * 4 + u
                            gs = GSIZES[g]
                            nc.tensor.transpose(pc[:, u * 128:u * 128 + gs],
                                                avout[0:gs, g * 192:g * 192 + 128],
                                                ident[0:gs, 0:gs])
                            nc.tensor.transpose(pe[0:64, u * 128:u * 128 + gs],
                                                avout[0:gs, g * 192 + 128:g * 192 + 192],
                                                ident[0:gs, 0:gs])
                        t0 = GOFFS[gp * 4]
                        if n == 4 and GSIZES[gp * 4 + 3] == 119:
                            nc.vector.tensor_copy(
                                aotA[:, t0:t0 + 476].rearrange("p (u c) -> p u c", c=119),
                                pc[:, :].rearrange("p (u c) -> p u c", c=128)[:, 0:4, 0:119])
                            nc.vector.tensor_copy(
                                aotB[64:128, t0:t0 + 476].rearrange("p (u c) -> p u c", c=119),
                                pe[0:64, :].rearrange("p (u c) -> p u c", c=128)[:, 0:4, 0:119])
                        else:
                            for u in range(n):
                                g = gp * 4 + u
                                gs = GSIZES[g]
                                gt = GOFFS[g]
                                nc.vector.tensor_copy(aotA[:, gt:gt + gs],
                                                      pc[:, u * 128:u * 128 + gs])
                                nc.vector.tensor_copy(aotB[64:128, gt:gt + gs],
                                                      pe[0:64, u * 128:u * 128 + gs])

                prev = {"aotA": aotA, "aotB": aotB, "r0": r0}

            emit_po(prev)

    nc.finalize()
    return nc


def kernel(x, W_qkv, b_qkv, key_rel, key_rel_diag, W_out, b_out):
    from concourse.bass_utils import run_bass_kernel_spmd

    x = np.ascontiguousarray(np.asarray(x, dtype=np.float32))
    consts, mgrp = _build_host_constants(
        _f32(W_qkv), _f32(b_qkv), _f32(key_rel), _f32(key_rel_diag),
        _f32(W_out), _f32(b_out))

    if "nc" not in _CACHED:
        _CACHED["nc"] = _build_bass()
    nc = _CACHED["nc"]

    # x_ext: [NCORES, KL, TOK] logical rows = 192 features + 8 mask rows,
    # then hi/lo split and DoubleRow fold-100 packing.
    xT = x.reshape(NCORES, BC * V, DIM).transpose(0, 2, 1)  # [NC, 192, TOK]
    mgrp_full = np.tile(mgrp, (1, NCHUNK))                  # [8, TOK]
    x_hi_l = []
    x_lo_l = []
    for k in range(NCORES):
        ext = np.concatenate([xT[k], mgrp_full], axis=0)    # [200, TOK]
        hi = _f32(_fp8(ext))
        lo = (ext - hi) * 16.0
        lo[DIM:KL] = 0.0
        x_hi_l.append(_pack_dr(_fp8(hi)))
        x_lo_l.append(_pack_dr(_fp8(lo)))

    in_maps = [dict(consts, x_hi=x_hi_l[k], x_lo=x_lo_l[k])
               for k in range(NCORES)]
    res = run_bass_kernel_spmd(nc, in_maps, core_ids=list(range(NCORES)))
    _CACHED["last_result"] = res
    out = np.stack([res.results[k]["y"] for k in range(NCORES)], axis=0)
    out = out.reshape(B, V, DIM)
    # Value bias passes through normalized attention as a constant; fold it
    # (and b_out) exactly on the host. The q/k biases are zeros per the spec.
    bv = _f32(b_qkv)[2 * DIM:3 * DIM]
    bias = bv @ _f32(W_out) + _f32(b_out)
    if np.any(bias):
        out = out + bias
    return out
